# revision 1
# baseline (speedup 1.0000x reference)
"""AttnBlock (GroupNorm + single-head self-attention + residual) on 8 trn2 cores.

Problem: X [4, 512, 64, 64] f32. Per batch element: GroupNorm(32 groups), then
1x1-conv Q/K/V projections, softmax attention over n=h*w=4096 positions,
proj_out, residual add.

Sharding: 8 cores = 4 batch elements x 2 query-halves. Each core computes the
full GroupNorm + K/V for its batch element (duplicated within the pair) and
attention output for its 2048-query half.

Layout strategy (per core):
  Hn, K, Q kept channel-major [c, n] (c on partitions)  -> projections are
  natural matmuls.  S^T[k, q] = sum_c K[c,k] Q[c,q] computed with k on
  partitions so softmax sums reduce via a ones-vector matmul on the PE and
  Ho[q, c] = sum_k expS[k,q] V[k,c] accumulates flash-style in PSUM without
  ever materializing/transposing the 4096x4096 attention matrix.
  Softmax skips max-subtraction: |S*scale| < ~10 here, exp is safe in f32.

All big matmuls run in float32r (full PE rate at N=512, ~1.5e-4 rel err).

SBUF (208KB/partition) forces a two-pass GroupNorm: pass 1 streams X for
stats only; pass 2 re-reads X in halves, normalizes, and immediately
projects K (staged to DRAM scratch) and V.  Q likewise from the Xq input.
K is reloaded into SBUF for the attention phase once Hn is gone.
"""

import numpy as np

B, C, H, W = 4, 512, 64, 64
N = H * W            # 4096 keys per batch element
NQ = N // 2          # 2048 queries per core
CT = C // 128        # 4 channel tiles
NT = N // 128        # 32 key tiles
QC = NQ // 512       # 4 query chunks of 512
GROUPS = 32
GPT = GROUPS // CT   # 8 groups per 128-channel tile
GSZ = C // GROUPS    # 16 channels per group
EPS = 1e-5
SCALE = float(C) ** -0.5

_CACHE = {}


def _build(debug=False):
    from contextlib import ExitStack
    from concourse import bacc
    import concourse.mybir as mybir
    import concourse.tile as tile
    from concourse.masks import make_identity

    f32 = mybir.dt.float32
    f32r = mybir.dt.float32r
    AF = mybir.ActivationFunctionType
    OP = mybir.AluOpType

    nc = bacc.Bacc()
    X = nc.dram_tensor("X", [C, N], f32, kind="ExternalInput")
    Xq = nc.dram_tensor("Xq", [C, NQ], f32, kind="ExternalInput")
    wT = {
        nm: nc.dram_tensor(nm, [C, C], f32, kind="ExternalInput")
        for nm in ("wqT", "wkT", "wvT", "wpT")
    }
    vecs = {
        nm: nc.dram_tensor(nm, [C], f32, kind="ExternalInput")
        for nm in ("bq", "bk", "bpe", "gn_w", "gn_b")
    }
    gmat_d = nc.dram_tensor("gmat_d", [128, GPT], f32, kind="ExternalInput")
    ones2_d = nc.dram_tensor("ones2_d", [128, 2], f32, kind="ExternalInput")
    gmatT_d = nc.dram_tensor("gmatT_d", [GPT, 128], f32, kind="ExternalInput")
    out = nc.dram_tensor("out", [C, NQ], f32, kind="ExternalOutput")
    dbg = {}
    if debug:
        for nm, shp in [("dbg_scbi", [128, 2 * CT]), ("dbg_q", [128, 512]),
                        ("dbg_k", [128, 512]), ("dbg_v", [128, C]),
                        ("dbg_es", [128, 512]), ("dbg_sums", [128, 8]),
                        ("dbg_ho", [128, 512]), ("dbg_hoT", [128, 512]),
                        ("dbg_sraw", [128, 512])]:
            dbg[nm] = nc.dram_tensor(nm, shp, f32, kind="ExternalOutput")

    def col(v, ci):
        # [C] dram vector -> [128, 1] AP for channel tile ci
        return vecs[v][ci * 128:(ci + 1) * 128].rearrange("(p one) -> p one", one=1)

    def load_f32r(pool, stage_pool, dram_ap, shape, tag):
        """DMA f32 -> staging, DVE-convert -> f32r tile (real format change)."""
        st = stage_pool.tile(shape, f32, tag="ld_stage", name="ld_stage")
        nc.sync.dma_start(out=st, in_=dram_ap)
        t = pool.tile(shape, f32r, tag=tag, name=tag)
        nc.vector.tensor_copy(out=t, in_=st)
        return t

    # fp32r is an opaque on-chip format: every fp32r operand must be produced
    # by a compute-engine conversion (DVE copy), never by a bitcast DMA.

    with tile.TileContext(nc) as tc, ExitStack() as ctx:
        consts = ctx.enter_context(tc.tile_pool(name="consts", bufs=1))
        pp_acc = ctx.enter_context(tc.tile_pool(name="pp_acc", bufs=4, space="PSUM"))
        pp_sps = ctx.enter_context(tc.tile_pool(name="pp_sps", bufs=3, space="PSUM"))
        pp_sums = ctx.enter_context(tc.tile_pool(name="pp_sums", bufs=1, space="PSUM"))

        # ---- pass A: stream X quarters for GroupNorm statistics ----
        # (emitted FIRST so the X DMA triggers lead the queues)
        gst_cm = tc.tile_pool(name="gn_stats", bufs=2)
        gstats = gst_cm.__enter__()
        xst_cm = tc.tile_pool(name="xstream", bufs=3)
        xstream = xst_cm.__enter__()
        if True:
            rowst_all = gstats.tile([128, CT, 2], f32r, tag="rowst", name="rowst")
            with nc.named_scope("gn"):
                for ci in range(CT):
                    stats = gstats.tile([128, N // 512, 6], f32, tag="bnst",
                                        name="bnst")
                    for q4 in range(4):
                        xs = xstream.tile([128, N // 4], f32, tag="xs", name="xs")
                        eng = nc.gpsimd if (ci * 4 + q4) % 2 else nc.sync
                        eng.dma_start(
                            out=xs,
                            in_=X[ci * 128:(ci + 1) * 128,
                                  q4 * (N // 4):(q4 + 1) * (N // 4)])
                        for s in range(N // 4 // 512):
                            nc.vector.bn_stats(
                                out=stats[:, q4 * 2 + s, :],
                                in_=xs[:, s * 512:(s + 1) * 512])
                    mv = gstats.tile([128, 2], f32, tag="mv", name="mv")
                    nc.vector.bn_aggr(out=mv, in_=stats)
                    # rowstats = [mean, E[x^2]] ; E[x^2] = var + mean^2
                    nc.vector.tensor_copy(out=rowst_all[:, ci, 0:1],
                                          in_=mv[:, 0:1])
                    m2 = gstats.tile([128, 1], f32, tag="m2", name="m2")
                    nc.vector.tensor_mul(out=m2, in0=mv[:, 0:1], in1=mv[:, 0:1])
                    nc.vector.tensor_add(out=rowst_all[:, ci, 1:2],
                                         in0=mv[:, 1:2], in1=m2)


        # ---- constants ----
        ident = consts.tile([128, 128], f32, tag="ident", name="ident")
        make_identity(nc, ident)
        with tc.tile_pool(name="cstage", bufs=2) as cstage:
            gmat = load_f32r(consts, cstage, gmat_d[:, :], [128, GPT], "gmat")
            gmatT = load_f32r(consts, cstage, gmatT_d[:, :], [GPT, 128], "gmatT")
            ones_col = load_f32r(consts, cstage, ones2_d[:, :], [128, 2], "ones")
        eps_t = consts.tile([128, 1], f32, tag="eps", name="eps")
        nc.vector.memset(eps_t, EPS)
        vt = {}
        for nm in ("bq", "bk", "bpe", "gn_w", "gn_b"):
            vt[nm] = consts.tile([128, CT], f32, tag=nm, name=nm)
            nc.sync.dma_start(
                out=vt[nm], in_=vecs[nm].rearrange("(c p) -> p c", p=128))
        # per-row GN affine: hn = x * sc_all[:,ci] + bi_all[:,ci]
        sc_all = consts.tile([128, CT], f32, tag="sc_all", name="sc_all")
        bi_all = consts.tile([128, CT], f32, tag="bi_all", name="bi_all")
        # proj weights stay resident (needed at the very end)
        wpT_sb = []
        with tc.tile_pool(name="wstage", bufs=2) as wstage:
            for ci in range(CT):
                wpT_sb.append(load_f32r(
                    consts, wstage, wT["wpT"][ci * 128:(ci + 1) * 128, :],
                    [128, C], f"wpT{ci}"))

        q_sb = [consts.tile([128, NQ], f32r, tag=f"q{co}", name=f"q{co}")
                for co in range(CT)]
        v_sb = [consts.tile([128, C], f32r, tag=f"v{nt}", name=f"v{nt}")
                for nt in range(NT)]

        # GN is folded into the projections: K = (wk*sc) @ X + (wk@bi + bk),
        # V likewise with its bias routed through proj_out (softmax rows sum
        # to 1), Q likewise.  X itself only needs a format conversion (on the
        # otherwise-idle Scalar engine) and the stats chain gates only the
        # small weight-fold ops, not a full normalization pass over X.
        bi2 = consts.tile([128, CT, 2], f32r, tag="bi2", name="bi2")
        kb_sb = consts.tile([128, CT], f32, tag="kb_sb", name="kb_sb")
        qb_sb = consts.tile([128, CT], f32, tag="qb_sb", name="qb_sb")
        vb2 = consts.tile([128, CT, 2], f32r, tag="vb2", name="vb2")
        pbe = consts.tile([128, CT], f32, tag="pbe", name="pbe")


        with nc.named_scope("gn2"):
                # group-reduce 128 rows -> 8 groups -> broadcast, all ci at once
                gps = pp_sps.tile([GPT, CT, 2], f32, tag="s_ps", name="gps")
                nc.tensor.matmul(out=gps, lhsT=gmat,
                                 rhs=rowst_all.rearrange("p c two -> p (c two)"),
                                 start=True, stop=True)
                gsb = gstats.tile([GPT, CT * 2], f32r, tag="gsb", name="gsb")
                nc.vector.tensor_copy(out=gsb,
                                      in_=gps.rearrange("g c two -> g (c two)"))
                bps = pp_sps.tile([128, CT, 2], f32, tag="s_ps", name="bps")
                nc.tensor.matmul(out=bps, lhsT=gmatT, rhs=gsb,
                                 start=True, stop=True)
                gstat = gstats.tile([128, CT, 2], f32, tag="gstat", name="gstat")
                nc.scalar.mul(out=gstat, in_=bps, mul=1.0 / GSZ)

                means = gstat[:, :, 0:1].rearrange("p c one -> p (c one)")
                m2s = gstat[:, :, 1:2].rearrange("p c one -> p (c one)")
                var = gstats.tile([128, CT], f32, tag="var", name="var")
                mm_ = gstats.tile([128, CT], f32, tag="mm_", name="mm_")
                nc.vector.tensor_mul(out=mm_, in0=means, in1=means)
                nc.vector.tensor_sub(out=var, in0=m2s, in1=mm_)
                # rstd = 1/sqrt(var + eps)
                nc.scalar.activation(out=var, in_=var, func=AF.Sqrt,
                                     bias=eps_t, scale=1.0)
                rstd = gstats.tile([128, CT], f32, tag="rstd", name="rstd")
                nc.vector.reciprocal(out=rstd, in_=var)
                # sc = rstd * gn_w ; bi = gn_b - mean * sc
                nc.vector.tensor_mul(out=sc_all, in0=rstd, in1=vt["gn_w"])
                msc = gstats.tile([128, CT], f32, tag="msc", name="msc")
                nc.vector.tensor_mul(out=msc, in0=means, in1=sc_all)
                nc.vector.tensor_sub(out=bi_all, in0=vt["gn_b"], in1=msc)
                for ci in range(CT):
                    nc.vector.tensor_copy(
                        out=bi2[:, ci, :],
                        in_=bi_all[:, ci:ci + 1].to_broadcast((128, 2)))

        xst_cm.__exit__(None, None, None)
        gst_cm.__exit__(None, None, None)


        def bias_matvec(w_sb, rhs2, add_vec):
            """[128, CT] per-partition vector = w.T-chunks @ rhs2 (+add_vec)."""
            outt = consts.tile([128, CT], f32, tag=f"bv_{w_sb[0].tensor.name}",
                               name="bv")
            for co in range(CT):
                ps = pp_sps.tile([128, 2], f32, tag="s_ps", name="bv_ps")
                for ci in range(CT):
                    nc.tensor.matmul(
                        out=ps, lhsT=w_sb[ci][:, co * 128:(co + 1) * 128],
                        rhs=rhs2[:, ci, :],
                        start=(ci == 0), stop=(ci == CT - 1))
                if add_vec is not None:
                    nc.vector.tensor_add(out=outt[:, co:co + 1],
                                         in0=ps[:, 0:1],
                                         in1=add_vec[:, co:co + 1])
                else:
                    nc.vector.tensor_copy(out=outt[:, co:co + 1], in_=ps[:, 0:1])
            return outt

        def fold(w_sb):
            for ci in range(CT):
                nc.vector.tensor_scalar_mul(out=w_sb[ci], in0=w_sb[ci],
                                            scalar1=sc_all[:, ci:ci + 1])

        # K lives in SBUF from projection straight through attention.
        kpool = ctx.enter_context(tc.tile_pool(name="kpool", bufs=1))
        k_sb = [kpool.tile([128, N], f32r, tag=f"k{ci}", name=f"k{ci}")
                for ci in range(CT)]

        # ---- K/V/Q weight loads, bias matvecs, folds (overlap Q below) ----
        wkv_cm = tc.tile_pool(name="wkv", bufs=1)
        wkv = wkv_cm.__enter__()
        wk_sb, wv_sb = [], []
        for ci in range(CT):
            wk_sb.append(load_f32r(
                wkv, wkv, wT["wkT"][ci * 128:(ci + 1) * 128, :],
                [128, C], f"wk{ci}"))
            wv_sb.append(load_f32r(
                wkv, wkv, wT["wvT"][ci * 128:(ci + 1) * 128, :],
                [128, C], f"wv{ci}"))
        kb = bias_matvec(wk_sb, bi2, vt["bk"])
        nc.vector.tensor_copy(out=kb_sb, in_=kb)
        vb = bias_matvec(wv_sb, bi2, None)
        for ci in range(CT):
            nc.vector.tensor_copy(
                out=vb2[:, ci, :],
                in_=vb[:, ci:ci + 1].to_broadcast((128, 2)))
        pb = bias_matvec(wpT_sb, vb2, vt["bpe"])
        nc.vector.tensor_copy(out=pbe, in_=pb)
        fold(wk_sb)
        fold(wv_sb)

        # ---- Q (streamed Xq quarters) ----
        with tc.tile_pool(name="wq", bufs=1) as wqp:
            wq_sb = []
            for ci in range(CT):
                wq_sb.append(load_f32r(
                    wqp, wqp, wT["wqT"][ci * 128:(ci + 1) * 128, :],
                    [128, C], f"wq{ci}"))
            qb = bias_matvec(wq_sb, bi2, vt["bq"])
            nc.vector.tensor_copy(out=qb_sb, in_=qb)
            fold(wq_sb)
            with tc.tile_pool(name="hq_q", bufs=1) as hqpool:
                for qn in range(QC):
                    hq = []
                    for ci in range(CT):
                        t = hqpool.tile([128, 512], f32r, tag=f"xq{ci}",
                                        name=f"xq{ci}")
                        nc.gpsimd.dma_start(
                            out=t,
                            in_=Xq[ci * 128:(ci + 1) * 128,
                                   qn * 512:(qn + 1) * 512].bitcast(f32r))
                        nc.scalar.activation(out=t, in_=t.bitcast(f32),
                                             func=AF.Copy)
                        hq.append(t)
                    with nc.named_scope("qproj"):
                        for co in range(CT):
                            ps = pp_sps.tile([128, 512], f32, tag="s_ps",
                                             name="q_ps")
                            for ci in range(CT):
                                nc.tensor.matmul(
                                    out=ps,
                                    lhsT=wq_sb[ci][:, co * 128:(co + 1) * 128],
                                    rhs=hq[ci],
                                    start=(ci == 0), stop=(ci == CT - 1))
                            nc.vector.tensor_scalar_add(
                                out=q_sb[co][:, qn * 512:(qn + 1) * 512],
                                in0=ps, scalar1=qb_sb[:, co:co + 1])

        # ---- pass B: stream X eighths, project K (into SBUF) and V ----
        with tc.tile_pool(name="xb", bufs=2) as xbp:
            for e8 in range(8):
                ns = slice(e8 * 512, (e8 + 1) * 512)
                xb = []
                for ci in range(CT):
                    t = xbp.tile([128, 512], f32r, tag=f"xb{ci}", name=f"xb{ci}")
                    nc.gpsimd.dma_start(
                        out=t, in_=X[ci * 128:(ci + 1) * 128, ns].bitcast(f32r))
                    nc.scalar.activation(out=t, in_=t.bitcast(f32), func=AF.Copy)
                    xb.append(t)
                with nc.named_scope("kproj"):
                    for co in range(CT):
                        ps = pp_sps.tile([128, 512], f32, tag="s_ps", name="k_ps")
                        for ci in range(CT):
                            nc.tensor.matmul(
                                out=ps, lhsT=wk_sb[ci][:, co * 128:(co + 1) * 128],
                                rhs=xb[ci],
                                start=(ci == 0), stop=(ci == CT - 1))
                        nc.vector.tensor_scalar_add(out=k_sb[co][:, ns], in0=ps,
                                                    scalar1=kb_sb[:, co:co + 1])
                with nc.named_scope("vproj"):
                    for nt4 in range(4):
                        nt = e8 * 4 + nt4
                        ps = pp_sps.tile([128, 512], f32, tag="s_ps", name="v_ps")
                        for ci in range(CT):
                            nc.tensor.matmul(
                                out=ps,
                                lhsT=xb[ci][:, nt4 * 128:(nt4 + 1) * 128],
                                rhs=wv_sb[ci],
                                start=(ci == 0), stop=(ci == CT - 1))
                        nc.vector.tensor_copy(out=v_sb[nt], in_=ps)

        wkv_cm.__exit__(None, None, None)

        if debug:
            dt_ = consts.tile([128, 2 * CT], f32, tag="dbg1", name="dbg1")
            nc.vector.tensor_copy(out=dt_[:, :CT], in_=sc_all)
            nc.vector.tensor_copy(out=dt_[:, CT:], in_=bi_all)
            nc.sync.dma_start(out=dbg["dbg_scbi"][:, :], in_=dt_)
            dq = consts.tile([128, 512], f32, tag="dbg_q", name="dbg_q")
            nc.vector.tensor_copy(out=dq, in_=q_sb[0][:, :512])
            nc.sync.dma_start(out=dbg["dbg_q"][:, :], in_=dq)
            dv = consts.tile([128, C], f32, tag="dbg_v", name="dbg_v")
            nc.vector.tensor_copy(out=dv, in_=v_sb[0])
            nc.sync.dma_start(out=dbg["dbg_v"][:, :], in_=dv)

        # ---- attention ----
        with tc.tile_pool(name="work", bufs=2) as work:
            if debug:
                dk = work.tile([128, 512], f32, tag="dbg_k", name="dbg_k", bufs=1)
                nc.vector.tensor_copy(out=dk, in_=k_sb[0][:, :512])
                nc.sync.dma_start(out=dbg["dbg_k"][:, :], in_=dk)

            for qc in range(QC):
                qs = slice(qc * 512, (qc + 1) * 512)
                ho_ps = [pp_acc.tile([128, 512], f32, tag="acc", name="acc")
                         for _ in range(4)]
                sums_ps = pp_sums.tile([128, 8], f32, tag="sums", name="sums")
                nc.vector.memset(sums_ps, 0.0)
                def s_exp(kt):
                    s_ps = pp_sps.tile([128, 512], f32, tag="s_ps", name="s_ps")
                    with nc.named_scope("attn_s"):
                        for ci in range(CT):
                            nc.tensor.matmul(
                                out=s_ps, lhsT=k_sb[ci][:, kt * 128:(kt + 1) * 128],
                                rhs=q_sb[ci][:, qs],
                                start=(ci == 0), stop=(ci == CT - 1))
                    es = work.tile([128, 512], f32r, tag="es", name="es",
                                   bufs=4 if debug else 6)
                    nc.scalar.activation(out=es, in_=s_ps, func=AF.Exp, scale=SCALE)
                    return es

                es_next = s_exp(0)
                for kt in range(NT):
                    es = es_next
                    if kt + 1 < NT:
                        es_next = s_exp(kt + 1)
                    with nc.named_scope("attn_ho"):
                        for j in range(4):
                            nc.tensor.matmul(
                                out=ho_ps[j], lhsT=es[:, j * 128:(j + 1) * 128],
                                rhs=v_sb[kt],
                                start=(kt == 0), stop=(kt == NT - 1))
                            nc.tensor.matmul(
                                out=sums_ps[:, 2 * j:2 * j + 2],
                                lhsT=es[:, j * 128:(j + 1) * 128], rhs=ones_col,
                                start=False, stop=(kt == NT - 1),
                                skip_group_check=True)

                inv = work.tile([128, 8], f32, tag="inv", name="inv")
                nc.vector.reciprocal(out=inv, in_=sums_ps)
                if debug and qc == 0:
                    nc.sync.dma_start(out=dbg["dbg_sums"][:, :], in_=inv)

                hoT = [work.tile([128, 512], f32r, tag="hoT", name="hoT", bufs=4 if debug else 5)
                       for _ in range(CT)]
                scope_tail = nc.enter_named_scope("attn_tail", False)
                for j in range(4):
                    ho_sb = work.tile([128, 512], f32, tag="ho_sb", name="ho_sb", bufs=1 if debug else 2)
                    nc.vector.tensor_scalar_mul(out=ho_sb, in0=ho_ps[j],
                                                scalar1=inv[:, 2 * j:2 * j + 1])
                    if debug and qc == 0 and j == 0:
                        nc.sync.dma_start(out=dbg["dbg_ho"][:, :], in_=ho_sb)
                    for ci in range(CT):
                        tp = pp_sps.tile([128, 128], f32, tag="s_ps", name="tp")
                        nc.tensor.transpose(tp, ho_sb[:, ci * 128:(ci + 1) * 128],
                                            ident)
                        nc.vector.tensor_copy(
                            out=hoT[ci][:, j * 128:(j + 1) * 128], in_=tp)

                if debug and qc == 0:
                    dht = work.tile([128, 512], f32, tag="dbg_hoT", name="dbg_hoT", bufs=1)
                    nc.vector.tensor_copy(out=dht, in_=hoT[0])
                    nc.sync.dma_start(out=dbg["dbg_hoT"][:, :], in_=dht)
                nc.leave_named_scope("attn_tail", scope_tail[0], False)
                for co in range(CT):
                    ps = pp_sps.tile([128, 512], f32, tag="s_ps", name="pr_ps")
                    for ci in range(CT):
                        nc.tensor.matmul(
                            out=ps, lhsT=wpT_sb[ci][:, co * 128:(co + 1) * 128],
                            rhs=hoT[ci],
                            start=(ci == 0), stop=(ci == CT - 1))
                    xr = work.tile([128, 512], f32, tag="xr", name="xr", bufs=1 if debug else 2)
                    nc.sync.dma_start(out=xr, in_=Xq[co * 128:(co + 1) * 128, qs])
                    ot = work.tile([128, 512], f32, tag="ot", name="ot", bufs=1 if debug else 2)
                    nc.vector.tensor_scalar_add(out=ot, in0=ps,
                                                scalar1=pbe[:, co:co + 1])
                    nc.vector.tensor_add(out=ot, in0=ot, in1=xr)
                    nc.sync.dma_start(out=out[co * 128:(co + 1) * 128, qs], in_=ot)

    nc.compile()
    return nc


def _get_nc():
    if "nc" not in _CACHE:
        _CACHE["nc"] = _build()
    return _CACHE["nc"]


def _prep_in_maps(X, gn_w, gn_b, wq, bq, wk, bk, wv, bv, wp, bp):
    X = np.ascontiguousarray(np.asarray(X, dtype=np.float32))
    f = lambda a: np.ascontiguousarray(np.asarray(a, dtype=np.float32))
    gn_w, gn_b, bq, bk, bv, bp = map(f, (gn_w, gn_b, bq, bk, bv, bp))
    wq, wk, wv, wp = map(f, (wq, wk, wv, wp))

    Xf = X.reshape(B, C, N)
    bpe = wp @ bv + bp  # bv folded through proj_out (sum_k softmax == 1)
    wqT = np.ascontiguousarray(wq.T)
    wkT = np.ascontiguousarray(wk.T)
    wvT = np.ascontiguousarray(wv.T)
    wpT = np.ascontiguousarray(wp.T)

    gmat = np.zeros((128, GPT), np.float32)
    for g in range(GPT):
        gmat[g * GSZ:(g + 1) * GSZ, g] = 1.0
    gmatT = np.ascontiguousarray(gmat.T)

    in_maps = []
    for core in range(8):
        bi, half = core // 2, core % 2
        q0 = half * NQ
        Xb = Xf[bi]
        in_maps.append({
            "X": Xb,
            "Xq": np.ascontiguousarray(Xb[:, q0:q0 + NQ]),
            "wqT": wqT, "wkT": wkT, "wvT": wvT, "wpT": wpT,
            "bq": bq, "bk": bk, "bpe": bpe, "gn_w": gn_w, "gn_b": gn_b,
            "gmat_d": gmat, "gmatT_d": gmatT,
            "ones2_d": np.ones((128, 2), np.float32),
        })
    return in_maps


_last_in_maps = None


def kernel(X, gn_w, gn_b, wq, bq, wk, bk, wv, bv, wp, bp):
    from concourse.bass_utils import run_bass_kernel_spmd

    global _last_in_maps
    in_maps = _prep_in_maps(X, gn_w, gn_b, wq, bq, wk, bk, wv, bv, wp, bp)
    _last_in_maps = in_maps
    nc = _get_nc()
    res = run_bass_kernel_spmd(nc, in_maps, list(range(8)))
    out = np.empty((B, C, N), np.float32)
    for core in range(8):
        bi, half = core // 2, core % 2
        out[bi][:, half * NQ:(half + 1) * NQ] = res.results[core]["out"]
    return out.reshape(B, C, H, W)



# revision 2
# speedup vs baseline: 1.3378x; 1.3378x over previous
"""AttnBlock (GroupNorm + single-head self-attention + residual) on 8 trn2 cores.

Problem: X [4, 512, 64, 64] f32. Per batch element: GroupNorm(32 groups), then
1x1-conv Q/K/V projections, softmax attention over n=h*w=4096 positions,
proj_out, residual add.

Sharding: 8 cores = 4 batch elements x 2 query-halves. Each core computes the
full GroupNorm + K/V for its batch element (duplicated within the pair) and
attention output for its 2048-query half.

v2 layout strategy (per core):
  X streams once from HBM as bf16 directly into resident SBUF tiles (stats via
  bn_stats on the same tiles, no second pass).  GN is folded into the
  projection weights as in v1.  K/Q are bf16 [c, n] (full PE rate); V is fp8
  e4m3 in DoubleRow pair layout [k, 2, c] so the PV matmul runs at 2 rows per
  cycle AND produces Ho already transposed [c, q] for proj_out (no PE
  transposes).  Softmax: es = exp(S*scale - Z) quantized to e4m3 (Z=4 keeps
  es in e4m3 range; numerically validated ~6e-3 rel err vs 2e-2 budget).
  Row sums come from a gpsimd f32 accumulation of the es tiles (removes the
  512 ones-matvec LDWEIGHTS of v1); 1/sum is folded in AFTER proj_out via a
  rank-1 broadcast matmul, since proj is linear in the per-query scale.
"""

import numpy as np

B, C, H, W = 4, 512, 64, 64
N = H * W            # 4096 keys per batch element
NQ = N // 2          # 2048 queries per core
CT = C // 128        # 4 channel tiles
NT = N // 128        # 32 key tiles
NP = NT // 2         # 16 key pair-tiles (DoubleRow)
QC = NQ // 512       # 4 query chunks of 512
GROUPS = 32
GPT = GROUPS // CT   # 8 groups per 128-channel tile
GSZ = C // GROUPS    # 16 channels per group
EPS = 1e-5
SCALE = float(C) ** -0.5
ZSHIFT = 4.0         # exp shift: es = exp(S*scale - Z); S*scale in ~[-7.3, 7.3]

_CACHE = {}


def _build(debug=False):
    from contextlib import ExitStack
    from concourse import bacc
    import concourse.mybir as mybir
    import concourse.tile as tile

    f32 = mybir.dt.float32
    f32r = mybir.dt.float32r
    bf16 = mybir.dt.bfloat16
    f8 = mybir.dt.float8e4
    AF = mybir.ActivationFunctionType
    DR = mybir.MatmulPerfMode.DoubleRow

    nc = bacc.Bacc()
    Xbf = nc.dram_tensor("Xbf", [C, N], bf16, kind="ExternalInput")
    X8 = nc.dram_tensor("X8", [2, 128, 2 * N], f8, kind="ExternalInput")
    Xq = nc.dram_tensor("Xq", [C, NQ], bf16, kind="ExternalInput")
    Xr = nc.dram_tensor("Xr", [C, NQ], f32, kind="ExternalInput")
    wT = {}
    for nm in ("wqT", "wkT", "wpT"):
        wT[nm] = nc.dram_tensor(nm, [C, C], bf16, kind="ExternalInput")
    wT["wvT"] = nc.dram_tensor("wvT", [C, C], f32, kind="ExternalInput")
    vecs = {
        nm: nc.dram_tensor(nm, [C], f32, kind="ExternalInput")
        for nm in ("bq", "bk", "bpe", "gn_w", "gn_b")
    }
    gmat_d = nc.dram_tensor("gmat_d", [128, GPT], f32, kind="ExternalInput")
    gmatT_d = nc.dram_tensor("gmatT_d", [GPT, 128], f32, kind="ExternalInput")
    out = nc.dram_tensor("out", [C, NQ], f32, kind="ExternalOutput")
    dbg = {}
    if debug:
        for nm, shp in [("dbg_scbi", [128, 2 * CT]), ("dbg_q", [128, 512]),
                        ("dbg_k", [128, 512]), ("dbg_v", [128, 2 * 512]),
                        ("dbg_es", [128, 2 * 512]), ("dbg_esum", [128, 512]),
                        ("dbg_hoT", [128, 512]), ("dbg_inv", [128, 512])]:
            dbg[nm] = nc.dram_tensor(nm, shp, f32, kind="ExternalOutput")

    def load_f32r(pool, stage_pool, dram_ap, shape, tag):
        """DMA f32 -> staging, DVE-convert -> f32r tile (real format change)."""
        st = stage_pool.tile(shape, f32, tag="ld_stage", name="ld_stage")
        nc.sync.dma_start(out=st, in_=dram_ap)
        t = pool.tile(shape, f32r, tag=tag, name=tag)
        nc.vector.tensor_copy(out=t, in_=st)
        return t

    with tile.TileContext(nc) as tc, ExitStack() as ctx:
        consts = ctx.enter_context(tc.tile_pool(name="consts", bufs=1))
        pp_acc = ctx.enter_context(tc.tile_pool(name="pp_acc", bufs=4, space="PSUM"))
        pp_sps = ctx.enter_context(tc.tile_pool(name="pp_sps", bufs=3, space="PSUM"))
        pp_tail = ctx.enter_context(tc.tile_pool(name="pp_tail", bufs=1, space="PSUM"))

        # persistent bf16 X cache [c-tile][128, N]
        x_bf = [consts.tile([128, N], bf16, tag=f"xbf{ci}", name=f"xbf{ci}")
                for ci in range(CT)]
        # persistent fp8 X cache in DoubleRow pair layout [128, 2, N]
        x8 = [consts.tile([128, 2, N], f8, tag=f"x8_{p}", name=f"x8_{p}")
              for p in range(2)]

        # ---- pass A: stream X (bf16) for GroupNorm statistics ----
        # (emitted FIRST so the X DMA triggers lead the queues)
        gst_cm = tc.tile_pool(name="gn_stats", bufs=2)
        gstats = gst_cm.__enter__()
        rowst_all = gstats.tile([128, CT, 2], f32r, tag="rowst", name="rowst")
        with nc.named_scope("gn"):
            for ci in range(CT):
                stats = gstats.tile([128, N // 512, 6], f32, tag="bnst",
                                    name="bnst")
                for q4 in range(4):
                    sl = slice(q4 * (N // 4), (q4 + 1) * (N // 4))
                    eng = nc.gpsimd if (ci * 4 + q4) % 2 else nc.sync
                    eng.dma_start(out=x_bf[ci][:, sl],
                                  in_=Xbf[ci * 128:(ci + 1) * 128, sl])
                    for s in range(N // 4 // 512):
                        cs = q4 * (N // 4) + s * 512
                        nc.vector.bn_stats(
                            out=stats[:, q4 * 2 + s, :],
                            in_=x_bf[ci][:, cs:cs + 512])
                mv = gstats.tile([128, 2], f32, tag="mv", name="mv")
                nc.vector.bn_aggr(out=mv, in_=stats)
                # rowstats = [mean, E[x^2]] ; E[x^2] = var + mean^2
                nc.vector.tensor_copy(out=rowst_all[:, ci, 0:1],
                                      in_=mv[:, 0:1])
                m2 = gstats.tile([128, 1], f32, tag="m2", name="m2")
                nc.vector.tensor_mul(out=m2, in0=mv[:, 0:1], in1=mv[:, 0:1])
                nc.vector.tensor_add(out=rowst_all[:, ci, 1:2],
                                     in0=mv[:, 1:2], in1=m2)

        # ---- early DMAs: fp8 X, weights, q-half, vectors (overlap stats) ----
        for p in range(2):
            nc.scalar.dma_start(out=x8[p].rearrange("p two n -> p (two n)"),
                                in_=X8[p, :, :])
        wq_sb = [consts.tile([128, C], bf16, tag=f"wq{ci}", name=f"wq{ci}")
                 for ci in range(CT)]
        wk_sb = [consts.tile([128, C], bf16, tag=f"wk{ci}", name=f"wk{ci}")
                 for ci in range(CT)]
        wp_sb = [consts.tile([128, C], bf16, tag=f"wp{ci}", name=f"wp{ci}")
                 for ci in range(CT)]
        for ci in range(CT):
            rs = slice(ci * 128, (ci + 1) * 128)
            nc.scalar.dma_start(out=wk_sb[ci], in_=wT["wkT"][rs, :])
            nc.scalar.dma_start(out=wq_sb[ci], in_=wT["wqT"][rs, :])
            nc.scalar.dma_start(out=wp_sb[ci], in_=wT["wpT"][rs, :])
        # wv stays f32 (staged) so the fold+fp8 conversion is single-rounding
        wv_st = [consts.tile([128, C], f32, tag=f"wvst{ci}", name=f"wvst{ci}")
                 for ci in range(CT)]
        for ci in range(CT):
            nc.scalar.dma_start(out=wv_st[ci],
                                in_=wT["wvT"][ci * 128:(ci + 1) * 128, :])
        # query-half bf16 input for Q projection
        xq_bf = [consts.tile([128, NQ], bf16, tag=f"xq{ci}", name=f"xq{ci}")
                 for ci in range(CT)]
        for ci in range(CT):
            nc.scalar.dma_start(out=xq_bf[ci],
                                in_=Xq[ci * 128:(ci + 1) * 128, :])
        vt = {}
        for nm in ("bq", "bk", "bpe", "gn_w", "gn_b"):
            vt[nm] = consts.tile([128, CT], f32, tag=nm, name=nm)
            nc.sync.dma_start(
                out=vt[nm], in_=vecs[nm].rearrange("(c p) -> p c", p=128))

        # ---- constants ----
        with tc.tile_pool(name="cstage", bufs=2) as cstage:
            gmat = load_f32r(consts, cstage, gmat_d[:, :], [128, GPT], "gmat")
            gmatT = load_f32r(consts, cstage, gmatT_d[:, :], [GPT, 128], "gmatT")
        eps_t = consts.tile([128, 1], f32, tag="eps", name="eps")
        nc.vector.memset(eps_t, EPS)
        zsh_t = consts.tile([128, 1], f32, tag="zsh", name="zsh")
        nc.vector.memset(zsh_t, -ZSHIFT)
        ones_col = consts.tile([128, 1], bf16, tag="ones_c", name="ones_c")
        nc.vector.memset(ones_col, 1.0)
        ones_row = consts.tile([1, 128], bf16, tag="ones_r", name="ones_r")
        nc.vector.memset(ones_row, 1.0)

        # per-row GN affine: hn = x * sc_all[:,ci] + bi_all[:,ci]
        sc_all = consts.tile([128, CT], f32, tag="sc_all", name="sc_all")
        bi_all = consts.tile([128, CT], f32, tag="bi_all", name="bi_all")
        bi2 = consts.tile([128, CT, 2], bf16, tag="bi2", name="bi2")
        kb_sb = consts.tile([128, CT], f32, tag="kb_sb", name="kb_sb")
        qb_sb = consts.tile([128, CT], f32, tag="qb_sb", name="qb_sb")
        vb2 = consts.tile([128, CT, 2], bf16, tag="vb2", name="vb2")
        pbe = consts.tile([128, CT], f32, tag="pbe", name="pbe")

        with nc.named_scope("gn2"):
            # group-reduce 128 rows -> 8 groups -> broadcast, all ci at once
            gps = pp_sps.tile([GPT, CT, 2], f32, tag="s_ps", name="gps")
            nc.tensor.matmul(out=gps, lhsT=gmat,
                             rhs=rowst_all.rearrange("p c two -> p (c two)"),
                             start=True, stop=True)
            gsb = gstats.tile([GPT, CT * 2], f32r, tag="gsb", name="gsb")
            nc.vector.tensor_copy(out=gsb,
                                  in_=gps.rearrange("g c two -> g (c two)"))
            bps = pp_sps.tile([128, CT, 2], f32, tag="s_ps", name="bps")
            nc.tensor.matmul(out=bps, lhsT=gmatT, rhs=gsb,
                             start=True, stop=True)
            gstat = gstats.tile([128, CT, 2], f32, tag="gstat", name="gstat")
            nc.scalar.mul(out=gstat, in_=bps, mul=1.0 / GSZ)

            means = gstat[:, :, 0:1].rearrange("p c one -> p (c one)")
            m2s = gstat[:, :, 1:2].rearrange("p c one -> p (c one)")
            var = gstats.tile([128, CT], f32, tag="var", name="var")
            mm_ = gstats.tile([128, CT], f32, tag="mm_", name="mm_")
            nc.vector.tensor_mul(out=mm_, in0=means, in1=means)
            nc.vector.tensor_sub(out=var, in0=m2s, in1=mm_)
            # rstd = 1/sqrt(var + eps)
            nc.scalar.activation(out=var, in_=var, func=AF.Sqrt,
                                 bias=eps_t, scale=1.0)
            rstd = gstats.tile([128, CT], f32, tag="rstd", name="rstd")
            nc.vector.reciprocal(out=rstd, in_=var)
            # sc = rstd * gn_w ; bi = gn_b - mean * sc
            nc.vector.tensor_mul(out=sc_all, in0=rstd, in1=vt["gn_w"])
            msc = gstats.tile([128, CT], f32, tag="msc", name="msc")
            nc.vector.tensor_mul(out=msc, in0=means, in1=sc_all)
            nc.vector.tensor_sub(out=bi_all, in0=vt["gn_b"], in1=msc)
            for ci in range(CT):
                nc.vector.tensor_copy(
                    out=bi2[:, ci, :],
                    in_=bi_all[:, ci:ci + 1].to_broadcast((128, 2)))

        gst_cm.__exit__(None, None, None)

        def bias_matvec(w_sb, rhs2, add_vec, outname):
            """[128, CT] per-partition vector = w.T-chunks @ rhs2 (+add_vec)."""
            outt = consts.tile([128, CT], f32, tag=outname, name=outname)
            for co in range(CT):
                ps = pp_sps.tile([128, 2], f32, tag="s_ps", name="bv_ps")
                for ci in range(CT):
                    nc.tensor.matmul(
                        out=ps, lhsT=w_sb[ci][:, co * 128:(co + 1) * 128],
                        rhs=rhs2[:, ci, :],
                        start=(ci == 0), stop=(ci == CT - 1))
                if add_vec is not None:
                    nc.vector.tensor_add(out=outt[:, co:co + 1],
                                         in0=ps[:, 0:1],
                                         in1=add_vec[:, co:co + 1])
                else:
                    nc.vector.tensor_copy(out=outt[:, co:co + 1], in_=ps[:, 0:1])
            return outt

        # K/Q bias matvecs on UNFOLDED weights, then fold in place
        kb = bias_matvec(wk_sb, bi2, vt["bk"], "kb_t")
        nc.vector.tensor_copy(out=kb_sb, in_=kb)
        qb = bias_matvec(wq_sb, bi2, vt["bq"], "qb_t")
        nc.vector.tensor_copy(out=qb_sb, in_=qb)
        # V bias via a temporary bf16 copy of unfolded wv
        with tc.tile_pool(name="wvbf", bufs=1) as wvbfp:
            wv_bf = []
            for ci in range(CT):
                t = wvbfp.tile([128, C], bf16, tag=f"wvbf{ci}", name=f"wvbf{ci}")
                nc.vector.tensor_copy(out=t, in_=wv_st[ci])
                wv_bf.append(t)
            vb = bias_matvec(wv_bf, bi2, None, "vb_t")
            for ci in range(CT):
                nc.vector.tensor_copy(
                    out=vb2[:, ci, :],
                    in_=vb[:, ci:ci + 1].to_broadcast((128, 2)))
            pb = bias_matvec(wp_sb, vb2, vt["bpe"], "pb_t")
            nc.vector.tensor_copy(out=pbe, in_=pb)
        # folds: wk/wq in place (bf16), wv -> fp8 pair tiles
        for ci in range(CT):
            nc.vector.tensor_scalar_mul(out=wk_sb[ci], in0=wk_sb[ci],
                                        scalar1=sc_all[:, ci:ci + 1])
            nc.vector.tensor_scalar_mul(out=wq_sb[ci], in0=wq_sb[ci],
                                        scalar1=sc_all[:, ci:ci + 1])
        wv8 = [consts.tile([128, 2, C], f8, tag=f"wv8_{p}", name=f"wv8_{p}")
               for p in range(2)]
        for ci in range(CT):
            nc.vector.tensor_scalar_mul(out=wv8[ci // 2][:, ci % 2, :],
                                        in0=wv_st[ci],
                                        scalar1=sc_all[:, ci:ci + 1])

        # K lives in SBUF from projection straight through attention.
        k_sb = [consts.tile([128, N], bf16, tag=f"k{ci}", name=f"k{ci}")
                for ci in range(CT)]
        q_sb = [consts.tile([128, NQ], bf16, tag=f"q{co}", name=f"q{co}")
                for co in range(CT)]
        v8 = [consts.tile([128, 2, 512], f8, tag=f"v8_{p}", name=f"v8_{p}")
              for p in range(NP)]

        # ---- K projection (bf16): K[co, n] = sum_ci wkf[ci].T @ x_bf[ci] ----
        with nc.named_scope("kproj"):
            for e8 in range(8):
                ns = slice(e8 * 512, (e8 + 1) * 512)
                for co in range(CT):
                    ps = pp_sps.tile([128, 512], f32, tag="s_ps", name="k_ps")
                    for ci in range(CT):
                        nc.tensor.matmul(
                            out=ps, lhsT=wk_sb[ci][:, co * 128:(co + 1) * 128],
                            rhs=x_bf[ci][:, ns],
                            start=(ci == 0), stop=(ci == CT - 1))
                    nc.vector.tensor_scalar_add(out=k_sb[co][:, ns], in0=ps,
                                                scalar1=kb_sb[:, co:co + 1])
        # ---- V projection (fp8 DoubleRow): V[nt, c] then store [k,2,c] ----
        with nc.named_scope("vproj"):
            for nt in range(NT):
                ps = pp_sps.tile([128, 512], f32, tag="s_ps", name="v_ps")
                for p in range(2):
                    nc.tensor.matmul(
                        out=ps,
                        lhsT=x8[p][:, :, nt * 128:(nt + 1) * 128],
                        rhs=wv8[p],
                        start=(p == 0), stop=(p == 1), perf_mode=DR)
                nc.vector.tensor_copy(out=v8[nt // 2][:, nt % 2, :], in_=ps)
        # ---- Q projection (bf16) over this core's half ----
        with nc.named_scope("qproj"):
            for qn in range(QC):
                qs = slice(qn * 512, (qn + 1) * 512)
                for co in range(CT):
                    ps = pp_sps.tile([128, 512], f32, tag="s_ps", name="q_ps")
                    for ci in range(CT):
                        nc.tensor.matmul(
                            out=ps, lhsT=wq_sb[ci][:, co * 128:(co + 1) * 128],
                            rhs=xq_bf[ci][:, qs],
                            start=(ci == 0), stop=(ci == CT - 1))
                    nc.vector.tensor_scalar_add(out=q_sb[co][:, qs], in0=ps,
                                                scalar1=qb_sb[:, co:co + 1])

        if debug:
            dt_ = consts.tile([128, 2 * CT], f32, tag="dbg1", name="dbg1")
            nc.vector.tensor_copy(out=dt_[:, :CT], in_=sc_all)
            nc.vector.tensor_copy(out=dt_[:, CT:], in_=bi_all)
            nc.sync.dma_start(out=dbg["dbg_scbi"][:, :], in_=dt_)
            dq = consts.tile([128, 512], f32, tag="dbg_q", name="dbg_q")
            nc.vector.tensor_copy(out=dq, in_=q_sb[0][:, :512])
            nc.sync.dma_start(out=dbg["dbg_q"][:, :], in_=dq)
            dk = consts.tile([128, 512], f32, tag="dbg_k", name="dbg_k")
            nc.vector.tensor_copy(out=dk, in_=k_sb[0][:, :512])
            nc.sync.dma_start(out=dbg["dbg_k"][:, :], in_=dk)
            dv = consts.tile([128, 2 * 512], f32, tag="dbg_v", name="dbg_v")
            nc.vector.tensor_copy(
                out=dv, in_=v8[0].rearrange("p two n -> p (two n)"))
            nc.sync.dma_start(out=dbg["dbg_v"][:, :], in_=dv)

        # ---- attention ----
        with tc.tile_pool(name="work", bufs=2) as work:
            for qc in range(QC):
                qs = slice(qc * 512, (qc + 1) * 512)
                hoT_ps = [pp_acc.tile([128, 512], f32, tag="acc", name="acc")
                          for _ in range(CT)]
                esum = work.tile([128, 512], f32, tag="esum", name="esum",
                                 bufs=2)

                def es_pair(p):
                    ep = work.tile([128, 2, 512], f8, tag="es", name="es",
                                   bufs=4)
                    for half in range(2):
                        kt = 2 * p + half
                        s_ps = pp_sps.tile([128, 512], f32, tag="s_ps",
                                           name="s_ps")
                        with nc.named_scope("attn_s"):
                            for ci in range(CT):
                                nc.tensor.matmul(
                                    out=s_ps,
                                    lhsT=k_sb[ci][:, kt * 128:(kt + 1) * 128],
                                    rhs=q_sb[ci][:, qs],
                                    start=(ci == 0), stop=(ci == CT - 1))
                        nc.scalar.activation(out=ep[:, half, :], in_=s_ps,
                                             func=AF.Exp, scale=SCALE,
                                             bias=zsh_t)
                    return ep

                ep_next = es_pair(0)
                for p in range(NP):
                    ep = ep_next
                    if p + 1 < NP:
                        ep_next = es_pair(p + 1)
                    with nc.named_scope("attn_ho"):
                        for cj in range(CT):
                            nc.tensor.matmul(
                                out=hoT_ps[cj],
                                lhsT=v8[p][:, :, cj * 128:(cj + 1) * 128],
                                rhs=ep,
                                start=(p == 0), stop=(p == NP - 1),
                                perf_mode=DR)
                    # softmax denominators on gpsimd (f32 += fp8 pair)
                    if p == 0:
                        nc.gpsimd.tensor_copy(out=esum, in_=ep[:, 0, :])
                    else:
                        nc.gpsimd.tensor_add(out=esum, in0=esum,
                                             in1=ep[:, 0, :])
                    nc.gpsimd.tensor_add(out=esum, in0=esum, in1=ep[:, 1, :])

                # tail: sums -> inv -> broadcast; proj on unnormalized hoT
                scope_tail = nc.enter_named_scope("attn_tail", False)
                esum_bf = work.tile([128, 512], bf16, tag="esum_bf",
                                    name="esum_bf", bufs=2)
                nc.vector.tensor_copy(out=esum_bf, in_=esum)
                sums_ps = pp_tail.tile([1, 512], f32, tag="t_ps", name="sums")
                nc.tensor.matmul(out=sums_ps, lhsT=ones_col, rhs=esum_bf,
                                 start=True, stop=True)
                inv_row = work.tile([1, 512], f32, tag="inv", name="inv",
                                    bufs=2)
                nc.vector.reciprocal(out=inv_row, in_=sums_ps)
                inv_bf = work.tile([1, 512], bf16, tag="inv_bf", name="inv_bf",
                                   bufs=2)
                nc.vector.tensor_copy(out=inv_bf, in_=inv_row)
                invb_ps = pp_tail.tile([128, 512], f32, tag="t_ps", name="invb")
                nc.tensor.matmul(out=invb_ps, lhsT=ones_row, rhs=inv_bf,
                                 start=True, stop=True)
                invb = work.tile([128, 512], f32, tag="invb", name="invb",
                                 bufs=2)
                nc.vector.tensor_copy(out=invb, in_=invb_ps)
                hoT_sb = [work.tile([128, 512], bf16, tag="hoT", name="hoT",
                                    bufs=5) for _ in range(CT)]
                for cj in range(CT):
                    nc.vector.tensor_copy(out=hoT_sb[cj], in_=hoT_ps[cj])
                if debug and qc == 0:
                    de = work.tile([128, 2 * 512], f32, tag="dbg_es",
                                   name="dbg_es", bufs=1)
                    nc.vector.tensor_copy(
                        out=de, in_=ep.rearrange("p two n -> p (two n)"))
                    nc.sync.dma_start(out=dbg["dbg_es"][:, :], in_=de)
                    nc.sync.dma_start(out=dbg["dbg_esum"][:, :], in_=esum)
                    dh = work.tile([128, 512], f32, tag="dbg_hoT",
                                   name="dbg_hoT", bufs=1)
                    nc.vector.tensor_copy(out=dh, in_=hoT_sb[0])
                    nc.sync.dma_start(out=dbg["dbg_hoT"][:, :], in_=dh)
                    nc.sync.dma_start(out=dbg["dbg_inv"][:, :], in_=invb)
                nc.leave_named_scope("attn_tail", scope_tail[0], False)

                for co in range(CT):
                    ps = pp_sps.tile([128, 512], f32, tag="s_ps", name="pr_ps")
                    for ci in range(CT):
                        nc.tensor.matmul(
                            out=ps, lhsT=wp_sb[ci][:, co * 128:(co + 1) * 128],
                            rhs=hoT_sb[ci],
                            start=(ci == 0), stop=(ci == CT - 1))
                    xr = work.tile([128, 512], f32, tag="xr", name="xr", bufs=2)
                    nc.sync.dma_start(out=xr,
                                      in_=Xr[co * 128:(co + 1) * 128, qs])
                    ot = work.tile([128, 512], f32, tag="ot", name="ot", bufs=2)
                    nc.vector.tensor_mul(out=ot, in0=ps, in1=invb)
                    nc.vector.tensor_scalar_add(out=ot, in0=ot,
                                                scalar1=pbe[:, co:co + 1])
                    nc.vector.tensor_add(out=ot, in0=ot, in1=xr)
                    nc.sync.dma_start(out=out[co * 128:(co + 1) * 128, qs],
                                      in_=ot)

    nc.compile()
    return nc


def _get_nc():
    if "nc" not in _CACHE:
        _CACHE["nc"] = _build()
    return _CACHE["nc"]


def _prep_in_maps(X, gn_w, gn_b, wq, bq, wk, bk, wv, bv, wp, bp):
    import ml_dtypes
    bfl = ml_dtypes.bfloat16
    e4 = ml_dtypes.float8_e4m3

    X = np.ascontiguousarray(np.asarray(X, dtype=np.float32))
    f = lambda a: np.ascontiguousarray(np.asarray(a, dtype=np.float32))
    gn_w, gn_b, bq, bk, bv, bp = map(f, (gn_w, gn_b, bq, bk, bv, bp))
    wq, wk, wv, wp = map(f, (wq, wk, wv, wp))

    Xf = X.reshape(B, C, N)
    Xf_bf = Xf.astype(bfl)                       # [4, C, N]
    # fp8 X in DoubleRow pair layout: [pair, 128, (2, N)]
    X8v = Xf.reshape(B, 2, 2, 128, N).transpose(0, 1, 3, 2, 4)  # b,pair,p,i,n
    X8 = np.ascontiguousarray(X8v.reshape(B, 2, 128, 2 * N)).astype(e4)
    bpe = wp @ bv + bp  # bv folded through proj_out (softmax rows sum to 1)
    wqT = np.ascontiguousarray(wq.T).astype(bfl)
    wkT = np.ascontiguousarray(wk.T).astype(bfl)
    wpT = np.ascontiguousarray(wp.T).astype(bfl)
    wvT = np.ascontiguousarray(wv.T)

    gmat = np.zeros((128, GPT), np.float32)
    for g in range(GPT):
        gmat[g * GSZ:(g + 1) * GSZ, g] = 1.0
    gmatT = np.ascontiguousarray(gmat.T)

    in_maps = []
    for core in range(8):
        bi, half = core // 2, core % 2
        q0 = half * NQ
        in_maps.append({
            "Xbf": Xf_bf[bi],
            "X8": X8[bi],
            "Xq": np.ascontiguousarray(Xf_bf[bi][:, q0:q0 + NQ]),
            "Xr": np.ascontiguousarray(Xf[bi][:, q0:q0 + NQ]),
            "wqT": wqT, "wkT": wkT, "wvT": wvT, "wpT": wpT,
            "bq": bq, "bk": bk, "bpe": bpe, "gn_w": gn_w, "gn_b": gn_b,
            "gmat_d": gmat, "gmatT_d": gmatT,
        })
    return in_maps


_last_in_maps = None


def kernel(X, gn_w, gn_b, wq, bq, wk, bk, wv, bv, wp, bp):
    from concourse.bass_utils import run_bass_kernel_spmd

    global _last_in_maps
    in_maps = _prep_in_maps(X, gn_w, gn_b, wq, bq, wk, bk, wv, bv, wp, bp)
    _last_in_maps = in_maps
    nc = _get_nc()
    res = run_bass_kernel_spmd(nc, in_maps, list(range(8)))
    out = np.empty((B, C, N), np.float32)
    for core in range(8):
        bi, half = core // 2, core % 2
        out[bi][:, half * NQ:(half + 1) * NQ] = res.results[core]["out"]
    return out.reshape(B, C, H, W)


# revision 5
# speedup vs baseline: 1.3705x; 1.0244x over previous
"""AttnBlock (GroupNorm + single-head self-attention + residual) on 8 trn2 cores.

Problem: X [4, 512, 64, 64] f32. Per batch element: GroupNorm(32 groups), then
1x1-conv Q/K/V projections, softmax attention over n=h*w=4096 positions,
proj_out, residual add.

Sharding: 8 cores = 4 batch elements x 2 query-halves. Each core computes the
full GroupNorm + K/V for its batch element (duplicated within the pair) and
attention output for its 2048-query half.

v2 layout strategy (per core):
  X streams once from HBM (host pre-converted bf16 + fp8 copies) directly into
  resident SBUF tiles; GroupNorm stats run on the fp8 copy (2.1MB critical
  path).  GN is folded into the projection weights.  K/Q are bf16 [c, n] (full
  PE rate); V is fp8 e4m3 in DoubleRow pair layout [k, 2, c] so the PV matmul
  runs 256 keys per instruction AND produces Ho already transposed [c, q] for
  proj_out (no PE transposes).  Softmax: es = exp(S*scale - Z) quantized to
  e4m3 (Z=4; validated ~6e-3 rel err vs the 2e-2 budget).  Row sums come from
  a DVE f32 accumulation of the es tiles (no ones-matvec LDWEIGHTS); 1/sum is
  folded in AFTER proj_out via a rank-1 broadcast matmul, since proj is linear
  in the per-query scale.  Per-qc tails are emitted after the next qc's first
  attention matmuls so the tail's vector chain hides under PE work.
"""

import numpy as np

B, C, H, W = 4, 512, 64, 64
N = H * W            # 4096 keys per batch element
NQ = N // 2          # 2048 queries per core
CT = C // 128        # 4 channel tiles
NT = N // 128        # 32 key tiles
NP = NT // 2         # 16 key pair-tiles (DoubleRow)
QC = NQ // 512       # 4 query chunks of 512
GROUPS = 32
GPT = GROUPS // CT   # 8 groups per 128-channel tile
GSZ = C // GROUPS    # 16 channels per group
EPS = 1e-5
SCALE = float(C) ** -0.5
ZSHIFT = 4.0         # exp shift: es = exp(S*scale - Z); S*scale in ~[-7.3, 7.3]

_CACHE = {}


def _build(debug=False):
    from contextlib import ExitStack
    from concourse import bacc
    import concourse.mybir as mybir
    import concourse.tile as tile

    f32 = mybir.dt.float32
    f32r = mybir.dt.float32r
    bf16 = mybir.dt.bfloat16
    f8 = mybir.dt.float8e4
    AF = mybir.ActivationFunctionType
    DR = mybir.MatmulPerfMode.DoubleRow

    nc = bacc.Bacc()
    Xbf = nc.dram_tensor("Xbf", [C, N], bf16, kind="ExternalInput")
    X8 = nc.dram_tensor("X8", [2, 128, 2 * N], f8, kind="ExternalInput")
    Xq = nc.dram_tensor("Xq", [C, NQ], bf16, kind="ExternalInput")
    Xr = nc.dram_tensor("Xr", [C, NQ], f32, kind="ExternalInput")
    wT = {}
    for nm in ("wqT", "wkT", "wpT"):
        wT[nm] = nc.dram_tensor(nm, [C, C], bf16, kind="ExternalInput")
    wT["wvT"] = nc.dram_tensor("wvT", [C, C], f32, kind="ExternalInput")
    vecs = {
        nm: nc.dram_tensor(nm, [C], f32, kind="ExternalInput")
        for nm in ("bq", "bk", "bpe", "gn_w", "gn_b")
    }
    gmat_d = nc.dram_tensor("gmat_d", [128, GPT], f32, kind="ExternalInput")
    gmatT_d = nc.dram_tensor("gmatT_d", [GPT, 128], f32, kind="ExternalInput")
    out = nc.dram_tensor("out", [C, NQ], f32, kind="ExternalOutput")
    dbg = {}
    if debug:
        for nm, shp in [("dbg_scbi", [128, 2 * CT]), ("dbg_q", [128, 512]),
                        ("dbg_k", [128, 512]), ("dbg_v", [128, 2 * 512]),
                        ("dbg_esum", [128, 512]),
                        ("dbg_hoT", [128, 512]), ("dbg_inv", [128, 512])]:
            dbg[nm] = nc.dram_tensor(nm, shp, f32, kind="ExternalOutput")

    def load_f32r(pool, stage_pool, dram_ap, shape, tag):
        """DMA f32 -> staging, DVE-convert -> f32r tile (real format change)."""
        st = stage_pool.tile(shape, f32, tag="ld_stage", name="ld_stage")
        nc.sync.dma_start(out=st, in_=dram_ap)
        t = pool.tile(shape, f32r, tag=tag, name=tag)
        nc.vector.tensor_copy(out=t, in_=st)
        return t

    with tile.TileContext(nc) as tc, ExitStack() as ctx:
        consts = ctx.enter_context(tc.tile_pool(name="consts", bufs=1))
        pp_acc = ctx.enter_context(tc.tile_pool(name="pp_acc", bufs=4, space="PSUM"))
        pp_sps = ctx.enter_context(tc.tile_pool(name="pp_sps", bufs=2, space="PSUM"))
        pp_proj = ctx.enter_context(tc.tile_pool(name="pp_proj", bufs=2, space="PSUM"))

        # persistent fp8 X cache in DoubleRow pair layout [128, 2, N]
        x8 = [consts.tile([128, 2, N], f8, tag=f"x8_{p}", name=f"x8_{p}")
              for p in range(2)]
        # persistent bf16 X cache [c-tile][128, N]
        x_bf = [consts.tile([128, N], bf16, tag=f"xbf{ci}", name=f"xbf{ci}")
                for ci in range(CT)]

        front_cm = tc.tile_pool(name="front", bufs=1)
        front = front_cm.__enter__()

        # ---- pass A: stream X8 (fp8), run GroupNorm statistics on it ----
        # (emitted FIRST so these DMA triggers lead the queues)
        gst_cm = tc.tile_pool(name="gn_stats", bufs=2)
        gstats = gst_cm.__enter__()
        rowst_all = gstats.tile([128, CT, 2], f32r, tag="rowst", name="rowst")
        with nc.named_scope("gn"):
            for p2 in range(2):
                stats = [gstats.tile([128, 8, 6], f32, tag=f"bnst{i}",
                                     name=f"bnst{i}") for i in range(2)]
                for n8 in range(8):
                    sl = slice(n8 * 512, (n8 + 1) * 512)
                    eng = nc.gpsimd if (p2 * 8 + n8) % 2 else nc.sync
                    eng.dma_start(out=x8[p2][:, :, sl],
                                  in_=X8[p2, :, :].rearrange(
                                      "p (two n) -> p two n", two=2)[:, :, sl])
                    for i in range(2):
                        nc.vector.bn_stats(out=stats[i][:, n8, :],
                                           in_=x8[p2][:, i, sl])
                for i in range(2):
                    ci = 2 * p2 + i
                    mv = gstats.tile([128, 2], f32, tag="mv", name="mv")
                    nc.vector.bn_aggr(out=mv, in_=stats[i])
                    # rowstats = [mean, E[x^2]] ; E[x^2] = var + mean^2
                    nc.vector.tensor_copy(out=rowst_all[:, ci, 0:1],
                                          in_=mv[:, 0:1])
                    m2 = gstats.tile([128, 1], f32, tag="m2", name="m2")
                    nc.vector.tensor_mul(out=m2, in0=mv[:, 0:1], in1=mv[:, 0:1])
                    nc.vector.tensor_add(out=rowst_all[:, ci, 1:2],
                                         in0=mv[:, 1:2], in1=m2)

        # ---- early DMAs (split small, overlap stats) ----
        for ci in range(CT):
            rs = slice(ci * 128, (ci + 1) * 128)
            for n8 in range(8):
                sl = slice(n8 * 512, (n8 + 1) * 512)
                eng = nc.gpsimd if (ci * 8 + n8) % 2 else nc.sync
                eng.dma_start(out=x_bf[ci][:, sl], in_=Xbf[rs, sl])
        wq_sb = [consts.tile([128, C], bf16, tag=f"wq{ci}", name=f"wq{ci}")
                 for ci in range(CT)]
        wk_sb = [consts.tile([128, C], bf16, tag=f"wk{ci}", name=f"wk{ci}")
                 for ci in range(CT)]
        wp_sb = [consts.tile([128, C], bf16, tag=f"wp{ci}", name=f"wp{ci}")
                 for ci in range(CT)]
        # wv stays f32 (staged) so the fold+fp8 conversion is single-rounding
        wv_st = [front.tile([128, C], f32, tag=f"wvst{ci}", name=f"wvst{ci}")
                 for ci in range(CT)]
        for ci in range(CT):
            rs = slice(ci * 128, (ci + 1) * 128)
            for hh in range(2):
                hs = slice(hh * 256, (hh + 1) * 256)
                nc.scalar.dma_start(out=wk_sb[ci][:, hs], in_=wT["wkT"][rs, hs])
                nc.scalar.dma_start(out=wq_sb[ci][:, hs], in_=wT["wqT"][rs, hs])
                nc.scalar.dma_start(out=wv_st[ci][:, hs], in_=wT["wvT"][rs, hs])
                nc.scalar.dma_start(out=wp_sb[ci][:, hs], in_=wT["wpT"][rs, hs])
        # query-half bf16 input for Q projection
        xq_bf = [front.tile([128, NQ], bf16, tag=f"xq{ci}", name=f"xq{ci}")
                 for ci in range(CT)]
        for ci in range(CT):
            rs = slice(ci * 128, (ci + 1) * 128)
            for hh in range(2):
                hs = slice(hh * 1024, (hh + 1) * 1024)
                nc.scalar.dma_start(out=xq_bf[ci][:, hs], in_=Xq[rs, hs])
        vt = {}
        for nm in ("bq", "bk", "bpe", "gn_w", "gn_b"):
            vt[nm] = consts.tile([128, CT], f32, tag=nm, name=nm)
            nc.sync.dma_start(
                out=vt[nm], in_=vecs[nm].rearrange("(c p) -> p c", p=128))

        # ---- constants ----
        with tc.tile_pool(name="cstage", bufs=2) as cstage:
            gmat = load_f32r(consts, cstage, gmat_d[:, :], [128, GPT], "gmat")
            gmatT = load_f32r(consts, cstage, gmatT_d[:, :], [GPT, 128], "gmatT")
        eps_t = consts.tile([128, 1], f32, tag="eps", name="eps")
        nc.vector.memset(eps_t, EPS)
        zsh_t = consts.tile([128, 1], f32, tag="zsh", name="zsh")
        nc.vector.memset(zsh_t, -ZSHIFT)
        ones_col = consts.tile([128, 1], bf16, tag="ones_c", name="ones_c")
        nc.vector.memset(ones_col, 1.0)
        ones_row = consts.tile([1, 128], bf16, tag="ones_r", name="ones_r")
        nc.vector.memset(ones_row, 1.0)

        # per-row GN affine: hn = x * sc_all[:,ci] + bi_all[:,ci]
        sc_all = consts.tile([128, CT], f32, tag="sc_all", name="sc_all")
        bi_all = consts.tile([128, CT], f32, tag="bi_all", name="bi_all")
        bi2 = consts.tile([128, CT, 2], bf16, tag="bi2", name="bi2")
        kb_sb = consts.tile([128, CT], f32, tag="kb_sb", name="kb_sb")
        qb_sb = consts.tile([128, CT], f32, tag="qb_sb", name="qb_sb")
        vb2 = consts.tile([128, CT, 2], bf16, tag="vb2", name="vb2")
        pbe = consts.tile([128, CT], f32, tag="pbe", name="pbe")

        with nc.named_scope("gn2"):
            # group-reduce 128 rows -> 8 groups -> broadcast, all ci at once
            gps = pp_sps.tile([GPT, CT, 2], f32, tag="s_ps", name="gps")
            nc.tensor.matmul(out=gps, lhsT=gmat,
                             rhs=rowst_all.rearrange("p c two -> p (c two)"),
                             start=True, stop=True)
            gsb = gstats.tile([GPT, CT * 2], f32r, tag="gsb", name="gsb")
            nc.vector.tensor_copy(out=gsb,
                                  in_=gps.rearrange("g c two -> g (c two)"))
            bps = pp_sps.tile([128, CT, 2], f32, tag="s_ps", name="bps")
            nc.tensor.matmul(out=bps, lhsT=gmatT, rhs=gsb,
                             start=True, stop=True)
            gstat = gstats.tile([128, CT, 2], f32, tag="gstat", name="gstat")
            nc.scalar.mul(out=gstat, in_=bps, mul=1.0 / GSZ)

            means = gstat[:, :, 0:1].rearrange("p c one -> p (c one)")
            m2s = gstat[:, :, 1:2].rearrange("p c one -> p (c one)")
            var = gstats.tile([128, CT], f32, tag="var", name="var")
            mm_ = gstats.tile([128, CT], f32, tag="mm_", name="mm_")
            nc.vector.tensor_mul(out=mm_, in0=means, in1=means)
            nc.vector.tensor_sub(out=var, in0=m2s, in1=mm_)
            # rstd = 1/sqrt(var + eps)
            nc.scalar.activation(out=var, in_=var, func=AF.Sqrt,
                                 bias=eps_t, scale=1.0)
            rstd = gstats.tile([128, CT], f32, tag="rstd", name="rstd")
            nc.vector.reciprocal(out=rstd, in_=var)
            # sc = rstd * gn_w ; bi = gn_b - mean * sc
            nc.vector.tensor_mul(out=sc_all, in0=rstd, in1=vt["gn_w"])
            msc = gstats.tile([128, CT], f32, tag="msc", name="msc")
            nc.vector.tensor_mul(out=msc, in0=means, in1=sc_all)
            nc.vector.tensor_sub(out=bi_all, in0=vt["gn_b"], in1=msc)
            for ci in range(CT):
                nc.vector.tensor_copy(
                    out=bi2[:, ci, :],
                    in_=bi_all[:, ci:ci + 1].to_broadcast((128, 2)))

        gst_cm.__exit__(None, None, None)

        def bias_matvec(w_sb, rhs2, add_vec, outname):
            """[128, CT] per-partition vector = w.T-chunks @ rhs2 (+add_vec)."""
            outt = consts.tile([128, CT], f32, tag=outname, name=outname)
            for co in range(CT):
                ps = pp_sps.tile([128, 2], f32, tag="s_ps", name="bv_ps")
                for ci in range(CT):
                    nc.tensor.matmul(
                        out=ps, lhsT=w_sb[ci][:, co * 128:(co + 1) * 128],
                        rhs=rhs2[:, ci, :],
                        start=(ci == 0), stop=(ci == CT - 1))
                if add_vec is not None:
                    nc.vector.tensor_add(out=outt[:, co:co + 1],
                                         in0=ps[:, 0:1],
                                         in1=add_vec[:, co:co + 1])
                else:
                    nc.vector.tensor_copy(out=outt[:, co:co + 1], in_=ps[:, 0:1])
            return outt

        # K/Q bias matvecs on UNFOLDED weights, then fold in place
        kb = bias_matvec(wk_sb, bi2, vt["bk"], "kb_t")
        nc.vector.tensor_copy(out=kb_sb, in_=kb)
        qb = bias_matvec(wq_sb, bi2, vt["bq"], "qb_t")
        nc.vector.tensor_copy(out=qb_sb, in_=qb)
        # V bias via a temporary bf16 copy of unfolded wv
        with tc.tile_pool(name="wvbf", bufs=1) as wvbfp:
            wv_bf = []
            for ci in range(CT):
                t = wvbfp.tile([128, C], bf16, tag=f"wvbf{ci}", name=f"wvbf{ci}")
                nc.vector.tensor_copy(out=t, in_=wv_st[ci])
                wv_bf.append(t)
            vb = bias_matvec(wv_bf, bi2, None, "vb_t")
            for ci in range(CT):
                nc.vector.tensor_copy(
                    out=vb2[:, ci, :],
                    in_=vb[:, ci:ci + 1].to_broadcast((128, 2)))
            pb = bias_matvec(wp_sb, vb2, vt["bpe"], "pb_t")
            nc.vector.tensor_copy(out=pbe, in_=pb)
        # folds: wk/wq in place (bf16), wv -> fp8 pair tiles
        for ci in range(CT):
            nc.vector.tensor_scalar_mul(out=wk_sb[ci], in0=wk_sb[ci],
                                        scalar1=sc_all[:, ci:ci + 1])
            nc.vector.tensor_scalar_mul(out=wq_sb[ci], in0=wq_sb[ci],
                                        scalar1=sc_all[:, ci:ci + 1])
        wv8 = [consts.tile([128, 2, C], f8, tag=f"wv8_{p}", name=f"wv8_{p}")
               for p in range(2)]
        for ci in range(CT):
            nc.vector.tensor_scalar_mul(out=wv8[ci // 2][:, ci % 2, :],
                                        in0=wv_st[ci],
                                        scalar1=sc_all[:, ci:ci + 1])

        # K lives in SBUF from projection straight through attention.
        k_sb = [consts.tile([128, N], bf16, tag=f"k{ci}", name=f"k{ci}")
                for ci in range(CT)]
        q_sb = [consts.tile([128, NQ], bf16, tag=f"q{co}", name=f"q{co}")
                for co in range(CT)]
        v8 = [consts.tile([128, 2, 512], f8, tag=f"v8_{p}", name=f"v8_{p}")
              for p in range(NP)]

        # ---- K projection (bf16): K[co, n] = sum_ci wkf[ci].T @ x_bf[ci] ----
        with nc.named_scope("kproj"):
            for e8 in range(8):
                ns = slice(e8 * 512, (e8 + 1) * 512)
                for co in range(CT):
                    ps = pp_proj.tile([128, 512], f32, tag="p_ps", name="k_ps")
                    for ci in range(CT):
                        nc.tensor.matmul(
                            out=ps, lhsT=wk_sb[ci][:, co * 128:(co + 1) * 128],
                            rhs=x_bf[ci][:, ns],
                            start=(ci == 0), stop=(ci == CT - 1))
                    nc.vector.tensor_scalar_add(out=k_sb[co][:, ns], in0=ps,
                                                scalar1=kb_sb[:, co:co + 1])
        # ---- V projection (fp8 DoubleRow): V[nt, c] then store [k,2,c] ----
        with nc.named_scope("vproj"):
            for nt in range(NT):
                ps = pp_proj.tile([128, 512], f32, tag="p_ps", name="v_ps")
                for p in range(2):
                    nc.tensor.matmul(
                        out=ps,
                        lhsT=x8[p][:, :, nt * 128:(nt + 1) * 128],
                        rhs=wv8[p],
                        start=(p == 0), stop=(p == 1), perf_mode=DR)
                nc.vector.tensor_copy(out=v8[nt // 2][:, nt % 2, :], in_=ps)
        # ---- Q projection (bf16) over this core's half ----
        with nc.named_scope("qproj"):
            for qn in range(QC):
                qs = slice(qn * 512, (qn + 1) * 512)
                for co in range(CT):
                    ps = pp_proj.tile([128, 512], f32, tag="p_ps", name="q_ps")
                    for ci in range(CT):
                        nc.tensor.matmul(
                            out=ps, lhsT=wq_sb[ci][:, co * 128:(co + 1) * 128],
                            rhs=xq_bf[ci][:, qs],
                            start=(ci == 0), stop=(ci == CT - 1))
                    nc.vector.tensor_scalar_add(out=q_sb[co][:, qs], in0=ps,
                                                scalar1=qb_sb[:, co:co + 1])

        front_cm.__exit__(None, None, None)

        if debug:
            dt_ = consts.tile([128, 2 * CT], f32, tag="dbg1", name="dbg1")
            nc.vector.tensor_copy(out=dt_[:, :CT], in_=sc_all)
            nc.vector.tensor_copy(out=dt_[:, CT:], in_=bi_all)
            nc.sync.dma_start(out=dbg["dbg_scbi"][:, :], in_=dt_)
            dq = consts.tile([128, 512], f32, tag="dbg_q", name="dbg_q")
            nc.vector.tensor_copy(out=dq, in_=q_sb[0][:, :512])
            nc.sync.dma_start(out=dbg["dbg_q"][:, :], in_=dq)
            dk = consts.tile([128, 512], f32, tag="dbg_k", name="dbg_k")
            nc.vector.tensor_copy(out=dk, in_=k_sb[0][:, :512])
            nc.sync.dma_start(out=dbg["dbg_k"][:, :], in_=dk)
            dv = consts.tile([128, 2 * 512], f32, tag="dbg_v", name="dbg_v")
            nc.vector.tensor_copy(
                out=dv, in_=v8[0].rearrange("p two n -> p (two n)"))
            nc.sync.dma_start(out=dbg["dbg_v"][:, :], in_=dv)

        # ---- attention ----
        with tc.tile_pool(name="work", bufs=2) as work:
            pend_tail = [None]

            def make_tail(qc, qs, hoT_ps, esum, xr_tiles):
                def emit():
                    scope_tail = nc.enter_named_scope("attn_tail", False)
                    esum_bf = work.tile([128, 512], bf16, tag="esum_bf",
                                        name="esum_bf", bufs=2)
                    nc.vector.tensor_add(out=esum_bf, in0=esum[:, 0, :],
                                         in1=esum[:, 1, :])
                    sums_ps = pp_proj.tile([1, 512], f32, tag="p_ps",
                                           name="sums")
                    nc.tensor.matmul(out=sums_ps, lhsT=ones_col, rhs=esum_bf,
                                     start=True, stop=True)
                    inv_row = work.tile([1, 512], f32, tag="inv", name="inv",
                                        bufs=2)
                    nc.vector.reciprocal(out=inv_row, in_=sums_ps)
                    inv_bf = work.tile([1, 512], bf16, tag="inv_bf",
                                       name="inv_bf", bufs=2)
                    nc.vector.tensor_copy(out=inv_bf, in_=inv_row)
                    invb_ps = pp_proj.tile([128, 512], f32, tag="p_ps",
                                           name="invb")
                    nc.tensor.matmul(out=invb_ps, lhsT=ones_row, rhs=inv_bf,
                                     start=True, stop=True)
                    invb = work.tile([128, 512], f32, tag="invb", name="invb",
                                     bufs=2)
                    nc.vector.tensor_copy(out=invb, in_=invb_ps)
                    hoT_sb = [work.tile([128, 512], bf16, tag="hoT",
                                        name="hoT", bufs=5)
                              for _ in range(CT)]
                    for cj in range(CT):
                        nc.vector.tensor_copy(out=hoT_sb[cj], in_=hoT_ps[cj])
                    if debug and qc == 0:
                        de = work.tile([128, 512], f32, tag="dbg_esum",
                                       name="dbg_esum", bufs=1)
                        nc.vector.tensor_copy(out=de, in_=esum_bf)
                        nc.sync.dma_start(out=dbg["dbg_esum"][:, :], in_=de)
                        dh = work.tile([128, 512], f32, tag="dbg_hoT",
                                       name="dbg_hoT", bufs=1)
                        nc.vector.tensor_copy(out=dh, in_=hoT_sb[0])
                        nc.sync.dma_start(out=dbg["dbg_hoT"][:, :], in_=dh)
                        nc.sync.dma_start(out=dbg["dbg_inv"][:, :], in_=invb)
                    nc.leave_named_scope("attn_tail", scope_tail[0], False)

                    for co in range(CT):
                        ps = pp_proj.tile([128, 512], f32, tag="p_ps",
                                          name="pr_ps")
                        for ci in range(CT):
                            nc.tensor.matmul(
                                out=ps,
                                lhsT=wp_sb[ci][:, co * 128:(co + 1) * 128],
                                rhs=hoT_sb[ci],
                                start=(ci == 0), stop=(ci == CT - 1))
                        ot = work.tile([128, 512], f32, tag="ot", name="ot",
                                       bufs=2)
                        nc.vector.tensor_mul(out=ot, in0=ps, in1=invb)
                        nc.vector.tensor_scalar_add(out=ot, in0=ot,
                                                    scalar1=pbe[:, co:co + 1])
                        nc.vector.tensor_add(out=ot, in0=ot, in1=xr_tiles[co])
                        nc.sync.dma_start(
                            out=out[co * 128:(co + 1) * 128, qs], in_=ot)
                return emit

            for qc in range(QC):
                qs = slice(qc * 512, (qc + 1) * 512)
                hoT_ps = [pp_acc.tile([128, 512], f32, tag="acc", name="acc")
                          for _ in range(CT)]
                esum = work.tile([128, 2, 512], f32, tag="esum", name="esum",
                                 bufs=2)
                # residual prefetch for this qc's tail
                xr_tiles = []
                for co in range(CT):
                    xr = work.tile([128, 512], f32, tag="xr", name="xr",
                                   bufs=6)
                    nc.sync.dma_start(out=xr,
                                      in_=Xr[co * 128:(co + 1) * 128, qs])
                    xr_tiles.append(xr)

                def es_pair(p):
                    ep = work.tile([128, 2, 512], f8, tag="es", name="es",
                                   bufs=4)
                    for half in range(2):
                        kt = 2 * p + half
                        s_ps = pp_sps.tile([128, 512], f32, tag="s_ps",
                                           name="s_ps")
                        with nc.named_scope("attn_s"):
                            for ci in range(CT):
                                nc.tensor.matmul(
                                    out=s_ps,
                                    lhsT=k_sb[ci][:, kt * 128:(kt + 1) * 128],
                                    rhs=q_sb[ci][:, qs],
                                    start=(ci == 0), stop=(ci == CT - 1))
                        nc.scalar.activation(out=ep[:, half, :], in_=s_ps,
                                             func=AF.Exp, scale=SCALE,
                                             bias=zsh_t)
                    return ep

                ep_cur = es_pair(0)
                ep_nxt = es_pair(1)
                # previous qc's tail hides under this qc's first s-matmuls
                if pend_tail[0] is not None:
                    pend_tail[0]()
                    pend_tail[0] = None
                for p in range(NP):
                    with nc.named_scope("attn_ho"):
                        for cj in range(CT):
                            nc.tensor.matmul(
                                out=hoT_ps[cj],
                                lhsT=v8[p][:, :, cj * 128:(cj + 1) * 128],
                                rhs=ep_cur,
                                start=(p == 0), stop=(p == NP - 1),
                                perf_mode=DR)
                    # softmax denominators: DVE f32 += fp8, flat pair adds
                    epf = ep_cur.rearrange("p two n -> p (two n)")
                    esf = esum.rearrange("p two n -> p (two n)")
                    if p == 0:
                        nc.vector.tensor_copy(out=esf, in_=epf)
                    else:
                        nc.vector.tensor_add(out=esf, in0=esf, in1=epf)
                    ep_cur = ep_nxt
                    if p + 2 < NP:
                        ep_nxt = es_pair(p + 2)
                pend_tail[0] = make_tail(qc, qs, hoT_ps, esum, xr_tiles)
            pend_tail[0]()

    nc.compile()
    return nc


def _get_nc():
    if "nc" not in _CACHE:
        _CACHE["nc"] = _build()
    return _CACHE["nc"]


def _prep_in_maps(X, gn_w, gn_b, wq, bq, wk, bk, wv, bv, wp, bp):
    import ml_dtypes
    bfl = ml_dtypes.bfloat16
    e4 = ml_dtypes.float8_e4m3

    X = np.ascontiguousarray(np.asarray(X, dtype=np.float32))
    f = lambda a: np.ascontiguousarray(np.asarray(a, dtype=np.float32))
    gn_w, gn_b, bq, bk, bv, bp = map(f, (gn_w, gn_b, bq, bk, bv, bp))
    wq, wk, wv, wp = map(f, (wq, wk, wv, wp))

    Xf = X.reshape(B, C, N)
    Xf_bf = Xf.astype(bfl)                       # [4, C, N]
    # fp8 X in DoubleRow pair layout: [pair, 128, (2, N)]
    X8v = Xf.reshape(B, 2, 2, 128, N).transpose(0, 1, 3, 2, 4)  # b,pair,p,i,n
    X8 = np.ascontiguousarray(X8v.reshape(B, 2, 128, 2 * N)).astype(e4)
    bpe = wp @ bv + bp  # bv folded through proj_out (softmax rows sum to 1)
    wqT = np.ascontiguousarray(wq.T).astype(bfl)
    wkT = np.ascontiguousarray(wk.T).astype(bfl)
    wpT = np.ascontiguousarray(wp.T).astype(bfl)
    wvT = np.ascontiguousarray(wv.T)

    gmat = np.zeros((128, GPT), np.float32)
    for g in range(GPT):
        gmat[g * GSZ:(g + 1) * GSZ, g] = 1.0
    gmatT = np.ascontiguousarray(gmat.T)

    in_maps = []
    for core in range(8):
        bi, half = core // 2, core % 2
        q0 = half * NQ
        in_maps.append({
            "Xbf": Xf_bf[bi],
            "X8": X8[bi],
            "Xq": np.ascontiguousarray(Xf_bf[bi][:, q0:q0 + NQ]),
            "Xr": np.ascontiguousarray(Xf[bi][:, q0:q0 + NQ]),
            "wqT": wqT, "wkT": wkT, "wvT": wvT, "wpT": wpT,
            "bq": bq, "bk": bk, "bpe": bpe, "gn_w": gn_w, "gn_b": gn_b,
            "gmat_d": gmat, "gmatT_d": gmatT,
        })
    return in_maps


_last_in_maps = None


def kernel(X, gn_w, gn_b, wq, bq, wk, bk, wv, bv, wp, bp):
    from concourse.bass_utils import run_bass_kernel_spmd

    global _last_in_maps
    in_maps = _prep_in_maps(X, gn_w, gn_b, wq, bq, wk, bk, wv, bv, wp, bp)
    _last_in_maps = in_maps
    nc = _get_nc()
    res = run_bass_kernel_spmd(nc, in_maps, list(range(8)))
    out = np.empty((B, C, N), np.float32)
    for core in range(8):
        bi, half = core // 2, core % 2
        out[bi][:, half * NQ:(half + 1) * NQ] = res.results[core]["out"]
    return out.reshape(B, C, H, W)


# revision 7
# speedup vs baseline: 1.3716x; 1.0008x over previous
"""AttnBlock (GroupNorm + single-head self-attention + residual) on 8 trn2 cores.

Problem: X [4, 512, 64, 64] f32. Per batch element: GroupNorm(32 groups), then
1x1-conv Q/K/V projections, softmax attention over n=h*w=4096 positions,
proj_out, residual add.

Sharding: 8 cores = 4 batch elements x 2 query-halves. Each core computes the
full GroupNorm + K/V for its batch element (duplicated within the pair) and
attention output for its 2048-query half.

v2 layout strategy (per core):
  X streams once from HBM (host pre-converted bf16 + fp8 copies) directly into
  resident SBUF tiles; GroupNorm stats run on the fp8 copy (2.1MB critical
  path).  GN is folded into the projection weights.  K/Q are bf16 [c, n] (full
  PE rate); V is fp8 e4m3 in DoubleRow pair layout [k, 2, c] so the PV matmul
  runs 256 keys per instruction AND produces Ho already transposed [c, q] for
  proj_out (no PE transposes).  Softmax: es = exp(S*scale - Z) quantized to
  e4m3 (Z=4; validated ~6e-3 rel err vs the 2e-2 budget).  Row sums come from
  a DVE f32 accumulation of the es tiles (no ones-matvec LDWEIGHTS); 1/sum is
  folded in AFTER proj_out via a rank-1 broadcast matmul, since proj is linear
  in the per-query scale.  Per-qc tails are emitted after the next qc's first
  attention matmuls so the tail's vector chain hides under PE work.
"""

import numpy as np

B, C, H, W = 4, 512, 64, 64
N = H * W            # 4096 keys per batch element
NQ = N // 2          # 2048 queries per core
CT = C // 128        # 4 channel tiles
NT = N // 128        # 32 key tiles
NP = NT // 2         # 16 key pair-tiles (DoubleRow)
QC = NQ // 512       # 4 query chunks of 512
GROUPS = 32
GPT = GROUPS // CT   # 8 groups per 128-channel tile
GSZ = C // GROUPS    # 16 channels per group
EPS = 1e-5
SCALE = float(C) ** -0.5
ZSHIFT = 4.0         # exp shift: es = exp(S*scale - Z); S*scale in ~[-7.3, 7.3]

_CACHE = {}


def _build(debug=False):
    from contextlib import ExitStack
    from concourse import bacc
    import concourse.mybir as mybir
    import concourse.tile as tile

    f32 = mybir.dt.float32
    f32r = mybir.dt.float32r
    bf16 = mybir.dt.bfloat16
    f8 = mybir.dt.float8e4
    AF = mybir.ActivationFunctionType
    DR = mybir.MatmulPerfMode.DoubleRow

    nc = bacc.Bacc()
    Xbf = nc.dram_tensor("Xbf", [C, N], bf16, kind="ExternalInput")
    X8 = nc.dram_tensor("X8", [2, 128, 2 * N], f8, kind="ExternalInput")
    Xq = nc.dram_tensor("Xq", [C, NQ], bf16, kind="ExternalInput")
    Xr = nc.dram_tensor("Xr", [C, NQ], f32, kind="ExternalInput")
    wT = {}
    for nm in ("wqT", "wkT", "wpT"):
        wT[nm] = nc.dram_tensor(nm, [C, C], bf16, kind="ExternalInput")
    wp8_d = nc.dram_tensor("wp8_d", [2, 128, 2 * C], f8, kind="ExternalInput")
    wT["wvT"] = nc.dram_tensor("wvT", [C, C], f32, kind="ExternalInput")
    vecs = {
        nm: nc.dram_tensor(nm, [C], f32, kind="ExternalInput")
        for nm in ("bq", "bk", "bpe", "gn_w", "gn_b")
    }
    gmat_d = nc.dram_tensor("gmat_d", [128, GPT], f32, kind="ExternalInput")
    gmatT_d = nc.dram_tensor("gmatT_d", [GPT, 128], f32, kind="ExternalInput")
    out = nc.dram_tensor("out", [C, NQ], f32, kind="ExternalOutput")
    dbg = {}
    if debug:
        for nm, shp in [("dbg_scbi", [128, 2 * CT]), ("dbg_q", [128, 512]),
                        ("dbg_k", [128, 512]), ("dbg_v", [128, 2 * 512]),
                        ("dbg_esum", [128, 512]),
                        ("dbg_hoT", [128, 512]), ("dbg_inv", [128, 512])]:
            dbg[nm] = nc.dram_tensor(nm, shp, f32, kind="ExternalOutput")

    def load_f32r(pool, stage_pool, dram_ap, shape, tag):
        """DMA f32 -> staging, DVE-convert -> f32r tile (real format change)."""
        st = stage_pool.tile(shape, f32, tag="ld_stage", name="ld_stage")
        nc.sync.dma_start(out=st, in_=dram_ap)
        t = pool.tile(shape, f32r, tag=tag, name=tag)
        nc.vector.tensor_copy(out=t, in_=st)
        return t

    with tile.TileContext(nc) as tc, ExitStack() as ctx:
        consts = ctx.enter_context(tc.tile_pool(name="consts", bufs=1))
        pp_acc = ctx.enter_context(tc.tile_pool(name="pp_acc", bufs=4, space="PSUM"))
        pp_sps = ctx.enter_context(tc.tile_pool(name="pp_sps", bufs=2, space="PSUM"))
        pp_proj = ctx.enter_context(tc.tile_pool(name="pp_proj", bufs=2, space="PSUM"))

        # persistent fp8 X cache in DoubleRow pair layout [128, 2, N]
        x8 = [consts.tile([128, 2, N], f8, tag=f"x8_{p}", name=f"x8_{p}")
              for p in range(2)]
        # persistent bf16 X cache [c-tile][128, N]
        x_bf = [consts.tile([128, N], bf16, tag=f"xbf{ci}", name=f"xbf{ci}")
                for ci in range(CT)]

        front_cm = tc.tile_pool(name="front", bufs=1)
        front = front_cm.__enter__()

        # tiny high-priority DMAs first: gn2 matrices + bias vectors
        with tc.tile_pool(name="cstage", bufs=2) as cstage:
            gmat = load_f32r(consts, cstage, gmat_d[:, :], [128, GPT], "gmat")
            gmatT = load_f32r(consts, cstage, gmatT_d[:, :], [GPT, 128], "gmatT")
        vt = {}
        for nm in ("bq", "bk", "bpe", "gn_w", "gn_b"):
            vt[nm] = consts.tile([128, CT], f32, tag=nm, name=nm)
            nc.sync.dma_start(
                out=vt[nm], in_=vecs[nm].rearrange("(c p) -> p c", p=128))

        # ---- pass A: stream X8 (fp8), run GroupNorm statistics on it ----
        # (emitted FIRST so these DMA triggers lead the queues)
        gst_cm = tc.tile_pool(name="gn_stats", bufs=2)
        gstats = gst_cm.__enter__()
        rowst_all = gstats.tile([128, CT, 2], f32r, tag="rowst", name="rowst")
        with nc.named_scope("gn"):
            for p2 in range(2):
                stats = [gstats.tile([128, 8, 6], f32, tag=f"bnst{i}",
                                     name=f"bnst{i}") for i in range(2)]
                for n8 in range(8):
                    sl = slice(n8 * 512, (n8 + 1) * 512)
                    eng = nc.gpsimd if (p2 * 8 + n8) % 2 else nc.sync
                    eng.dma_start(out=x8[p2][:, :, sl],
                                  in_=X8[p2, :, :].rearrange(
                                      "p (two n) -> p two n", two=2)[:, :, sl])
                    for i in range(2):
                        nc.vector.bn_stats(out=stats[i][:, n8, :],
                                           in_=x8[p2][:, i, sl])
                for i in range(2):
                    ci = 2 * p2 + i
                    mv = gstats.tile([128, 2], f32, tag="mv", name="mv")
                    nc.vector.bn_aggr(out=mv, in_=stats[i])
                    # rowstats = [mean, E[x^2]] ; E[x^2] = var + mean^2
                    nc.vector.tensor_copy(out=rowst_all[:, ci, 0:1],
                                          in_=mv[:, 0:1])
                    m2 = gstats.tile([128, 1], f32, tag="m2", name="m2")
                    nc.vector.tensor_mul(out=m2, in0=mv[:, 0:1], in1=mv[:, 0:1])
                    nc.vector.tensor_add(out=rowst_all[:, ci, 1:2],
                                         in0=mv[:, 1:2], in1=m2)

        # ---- early DMAs (split small, overlap stats) ----
        for ci in range(CT):
            rs = slice(ci * 128, (ci + 1) * 128)
            for n8 in range(8):
                sl = slice(n8 * 512, (n8 + 1) * 512)
                eng = nc.gpsimd if (ci * 8 + n8) % 2 else nc.sync
                eng.dma_start(out=x_bf[ci][:, sl], in_=Xbf[rs, sl])
        wq_sb = [consts.tile([128, C], bf16, tag=f"wq{ci}", name=f"wq{ci}")
                 for ci in range(CT)]
        wk_sb = [consts.tile([128, C], bf16, tag=f"wk{ci}", name=f"wk{ci}")
                 for ci in range(CT)]
        wp8 = [consts.tile([128, 2, C], f8, tag=f"wp8_{p}", name=f"wp8_{p}")
               for p in range(2)]
        wp_sb = [front.tile([128, C], bf16, tag=f"wp{ci}", name=f"wp{ci}")
                 for ci in range(CT)]
        for p in range(2):
            nc.scalar.dma_start(
                out=wp8[p].rearrange("p two n -> p (two n)"), in_=wp8_d[p, :, :])
        # wv stays f32 (staged) so the fold+fp8 conversion is single-rounding
        wv_st = [front.tile([128, C], f32, tag=f"wvst{ci}", name=f"wvst{ci}")
                 for ci in range(CT)]
        for ci in range(CT):
            rs = slice(ci * 128, (ci + 1) * 128)
            for hh in range(2):
                hs = slice(hh * 256, (hh + 1) * 256)
                nc.scalar.dma_start(out=wk_sb[ci][:, hs], in_=wT["wkT"][rs, hs])
                nc.scalar.dma_start(out=wq_sb[ci][:, hs], in_=wT["wqT"][rs, hs])
                nc.scalar.dma_start(out=wv_st[ci][:, hs], in_=wT["wvT"][rs, hs])
                nc.scalar.dma_start(out=wp_sb[ci][:, hs], in_=wT["wpT"][rs, hs])  # bf16 copy for pbe matvec only
        # query-half bf16 input for Q projection
        xq_bf = [front.tile([128, NQ], bf16, tag=f"xq{ci}", name=f"xq{ci}")
                 for ci in range(CT)]
        for ci in range(CT):
            rs = slice(ci * 128, (ci + 1) * 128)
            for hh in range(2):
                hs = slice(hh * 1024, (hh + 1) * 1024)
                nc.scalar.dma_start(out=xq_bf[ci][:, hs], in_=Xq[rs, hs])

        eps_t = consts.tile([128, 1], f32, tag="eps", name="eps")
        nc.vector.memset(eps_t, EPS)
        zsh_t = consts.tile([128, 1], f32, tag="zsh", name="zsh")
        nc.vector.memset(zsh_t, -ZSHIFT)
        ones_col = consts.tile([128, 1], bf16, tag="ones_c", name="ones_c")
        nc.vector.memset(ones_col, 1.0)
        ones_row = consts.tile([1, 128], bf16, tag="ones_r", name="ones_r")
        nc.vector.memset(ones_row, 1.0)

        # per-row GN affine: hn = x * sc_all[:,ci] + bi_all[:,ci]
        sc_all = consts.tile([128, CT], f32, tag="sc_all", name="sc_all")
        bi_all = consts.tile([128, CT], f32, tag="bi_all", name="bi_all")
        bi2 = consts.tile([128, CT, 2], bf16, tag="bi2", name="bi2")
        kb_sb = consts.tile([128, CT], f32, tag="kb_sb", name="kb_sb")
        qb_sb = consts.tile([128, CT], f32, tag="qb_sb", name="qb_sb")
        vb2 = consts.tile([128, CT, 2], bf16, tag="vb2", name="vb2")
        pbe = consts.tile([128, CT], f32, tag="pbe", name="pbe")

        with nc.named_scope("gn2"):
            # group-reduce 128 rows -> 8 groups -> broadcast, all ci at once
            gps = pp_sps.tile([GPT, CT, 2], f32, tag="s_ps", name="gps")
            nc.tensor.matmul(out=gps, lhsT=gmat,
                             rhs=rowst_all.rearrange("p c two -> p (c two)"),
                             start=True, stop=True)
            gsb = gstats.tile([GPT, CT * 2], f32r, tag="gsb", name="gsb")
            nc.vector.tensor_copy(out=gsb,
                                  in_=gps.rearrange("g c two -> g (c two)"))
            bps = pp_sps.tile([128, CT, 2], f32, tag="s_ps", name="bps")
            nc.tensor.matmul(out=bps, lhsT=gmatT, rhs=gsb,
                             start=True, stop=True)
            gstat = gstats.tile([128, CT, 2], f32, tag="gstat", name="gstat")
            nc.scalar.mul(out=gstat, in_=bps, mul=1.0 / GSZ)

            means = gstat[:, :, 0:1].rearrange("p c one -> p (c one)")
            m2s = gstat[:, :, 1:2].rearrange("p c one -> p (c one)")
            var = gstats.tile([128, CT], f32, tag="var", name="var")
            mm_ = gstats.tile([128, CT], f32, tag="mm_", name="mm_")
            nc.vector.tensor_mul(out=mm_, in0=means, in1=means)
            nc.vector.tensor_sub(out=var, in0=m2s, in1=mm_)
            # rstd = 1/sqrt(var + eps)
            nc.scalar.activation(out=var, in_=var, func=AF.Sqrt,
                                 bias=eps_t, scale=1.0)
            rstd = gstats.tile([128, CT], f32, tag="rstd", name="rstd")
            nc.vector.reciprocal(out=rstd, in_=var)
            # sc = rstd * gn_w ; bi = gn_b - mean * sc
            nc.vector.tensor_mul(out=sc_all, in0=rstd, in1=vt["gn_w"])
            msc = gstats.tile([128, CT], f32, tag="msc", name="msc")
            nc.vector.tensor_mul(out=msc, in0=means, in1=sc_all)
            nc.vector.tensor_sub(out=bi_all, in0=vt["gn_b"], in1=msc)
            for ci in range(CT):
                nc.vector.tensor_copy(
                    out=bi2[:, ci, :],
                    in_=bi_all[:, ci:ci + 1].to_broadcast((128, 2)))

        gst_cm.__exit__(None, None, None)

        def bias_matvec(w_sb, rhs2, add_vec, outname):
            """[128, CT] per-partition vector = w.T-chunks @ rhs2 (+add_vec)."""
            outt = consts.tile([128, CT], f32, tag=outname, name=outname)
            for co in range(CT):
                ps = pp_sps.tile([128, 2], f32, tag="s_ps", name="bv_ps")
                for ci in range(CT):
                    nc.tensor.matmul(
                        out=ps, lhsT=w_sb[ci][:, co * 128:(co + 1) * 128],
                        rhs=rhs2[:, ci, :],
                        start=(ci == 0), stop=(ci == CT - 1))
                if add_vec is not None:
                    nc.vector.tensor_add(out=outt[:, co:co + 1],
                                         in0=ps[:, 0:1],
                                         in1=add_vec[:, co:co + 1])
                else:
                    nc.vector.tensor_copy(out=outt[:, co:co + 1], in_=ps[:, 0:1])
            return outt

        # K/Q bias matvecs on UNFOLDED weights, then fold in place
        kb = bias_matvec(wk_sb, bi2, vt["bk"], "kb_t")
        nc.vector.tensor_copy(out=kb_sb, in_=kb)
        qb = bias_matvec(wq_sb, bi2, vt["bq"], "qb_t")
        nc.vector.tensor_copy(out=qb_sb, in_=qb)
        # V bias via a temporary bf16 copy of unfolded wv
        with tc.tile_pool(name="wvbf", bufs=1) as wvbfp:
            wv_bf = []
            for ci in range(CT):
                t = wvbfp.tile([128, C], bf16, tag=f"wvbf{ci}", name=f"wvbf{ci}")
                nc.vector.tensor_copy(out=t, in_=wv_st[ci])
                wv_bf.append(t)
            vb = bias_matvec(wv_bf, bi2, None, "vb_t")
            for ci in range(CT):
                nc.vector.tensor_copy(
                    out=vb2[:, ci, :],
                    in_=vb[:, ci:ci + 1].to_broadcast((128, 2)))
            pb = bias_matvec(wp_sb, vb2, vt["bpe"], "pb_t")
            nc.vector.tensor_copy(out=pbe, in_=pb)
        # folds: wk/wq in place (bf16), wv -> fp8 pair tiles
        for ci in range(CT):
            nc.vector.tensor_scalar_mul(out=wk_sb[ci], in0=wk_sb[ci],
                                        scalar1=sc_all[:, ci:ci + 1])
            nc.vector.tensor_scalar_mul(out=wq_sb[ci], in0=wq_sb[ci],
                                        scalar1=sc_all[:, ci:ci + 1])
        wv8 = [consts.tile([128, 2, C], f8, tag=f"wv8_{p}", name=f"wv8_{p}")
               for p in range(2)]
        for ci in range(CT):
            nc.vector.tensor_scalar_mul(out=wv8[ci // 2][:, ci % 2, :],
                                        in0=wv_st[ci],
                                        scalar1=sc_all[:, ci:ci + 1])

        # K lives in SBUF from projection straight through attention.
        k_sb = [consts.tile([128, N], bf16, tag=f"k{ci}", name=f"k{ci}")
                for ci in range(CT)]
        q_sb = [consts.tile([128, NQ], bf16, tag=f"q{co}", name=f"q{co}")
                for co in range(CT)]
        v8 = [consts.tile([128, 2, 512], f8, tag=f"v8_{p}", name=f"v8_{p}")
              for p in range(NP)]

        # ---- K projection (bf16): K[co, n] = sum_ci wkf[ci].T @ x_bf[ci] ----
        with nc.named_scope("kproj"):
            for e8 in range(8):
                ns = slice(e8 * 512, (e8 + 1) * 512)
                for co in range(CT):
                    ps = pp_proj.tile([128, 512], f32, tag="p_ps", name="k_ps")
                    for ci in range(CT):
                        nc.tensor.matmul(
                            out=ps, lhsT=wk_sb[ci][:, co * 128:(co + 1) * 128],
                            rhs=x_bf[ci][:, ns],
                            start=(ci == 0), stop=(ci == CT - 1))
                    nc.vector.tensor_scalar_add(out=k_sb[co][:, ns], in0=ps,
                                                scalar1=kb_sb[:, co:co + 1])
        # ---- V projection (fp8 DoubleRow): V[nt, c] then store [k,2,c] ----
        with nc.named_scope("vproj"):
            for nt in range(NT):
                ps = pp_proj.tile([128, 512], f32, tag="p_ps", name="v_ps")
                for p in range(2):
                    nc.tensor.matmul(
                        out=ps,
                        lhsT=x8[p][:, :, nt * 128:(nt + 1) * 128],
                        rhs=wv8[p],
                        start=(p == 0), stop=(p == 1), perf_mode=DR)
                nc.scalar.copy(out=v8[nt // 2][:, nt % 2, :], in_=ps)
        # ---- Q projection (bf16) over this core's half ----
        with nc.named_scope("qproj"):
            for qn in range(QC):
                qs = slice(qn * 512, (qn + 1) * 512)
                for co in range(CT):
                    ps = pp_proj.tile([128, 512], f32, tag="p_ps", name="q_ps")
                    for ci in range(CT):
                        nc.tensor.matmul(
                            out=ps, lhsT=wq_sb[ci][:, co * 128:(co + 1) * 128],
                            rhs=xq_bf[ci][:, qs],
                            start=(ci == 0), stop=(ci == CT - 1))
                    nc.vector.tensor_scalar_add(out=q_sb[co][:, qs], in0=ps,
                                                scalar1=qb_sb[:, co:co + 1])

        front_cm.__exit__(None, None, None)

        if debug:
            dt_ = consts.tile([128, 2 * CT], f32, tag="dbg1", name="dbg1")
            nc.vector.tensor_copy(out=dt_[:, :CT], in_=sc_all)
            nc.vector.tensor_copy(out=dt_[:, CT:], in_=bi_all)
            nc.sync.dma_start(out=dbg["dbg_scbi"][:, :], in_=dt_)
            dq = consts.tile([128, 512], f32, tag="dbg_q", name="dbg_q")
            nc.vector.tensor_copy(out=dq, in_=q_sb[0][:, :512])
            nc.sync.dma_start(out=dbg["dbg_q"][:, :], in_=dq)
            dk = consts.tile([128, 512], f32, tag="dbg_k", name="dbg_k")
            nc.vector.tensor_copy(out=dk, in_=k_sb[0][:, :512])
            nc.sync.dma_start(out=dbg["dbg_k"][:, :], in_=dk)
            dv = consts.tile([128, 2 * 512], f32, tag="dbg_v", name="dbg_v")
            nc.vector.tensor_copy(
                out=dv, in_=v8[0].rearrange("p two n -> p (two n)"))
            nc.sync.dma_start(out=dbg["dbg_v"][:, :], in_=dv)

        # ---- attention ----
        with tc.tile_pool(name="work", bufs=2) as work:
            pend_tail = [None]

            def make_tail(qc, qs, hoT_ps, esum, xr_tiles):
                def emit():
                    scope_tail = nc.enter_named_scope("attn_tail", False)
                    esum_bf = work.tile([128, 512], bf16, tag="esum_bf",
                                        name="esum_bf", bufs=2)
                    nc.vector.tensor_add(out=esum_bf, in0=esum[:, 0, :],
                                         in1=esum[:, 1, :])
                    sums_ps = pp_proj.tile([1, 512], f32, tag="p_ps",
                                           name="sums")
                    nc.tensor.matmul(out=sums_ps, lhsT=ones_col, rhs=esum_bf,
                                     start=True, stop=True)
                    sums_bf = work.tile([1, 512], bf16, tag="sums_bf",
                                        name="sums_bf", bufs=2)
                    nc.vector.tensor_copy(out=sums_bf, in_=sums_ps)
                    sumb_ps = pp_proj.tile([128, 512], f32, tag="p_ps",
                                           name="sumb")
                    nc.tensor.matmul(out=sumb_ps, lhsT=ones_row, rhs=sums_bf,
                                     start=True, stop=True)
                    invb = work.tile([128, 512], f32, tag="invb", name="invb",
                                     bufs=2)
                    nc.vector.reciprocal(out=invb, in_=sumb_ps)
                    hoT8 = [work.tile([128, 2, 512], f8, tag="hoT",
                                       name="hoT", bufs=3) for _ in range(2)]
                    for cj in range(CT):
                        nc.vector.tensor_copy(out=hoT8[cj // 2][:, cj % 2, :],
                                              in_=hoT_ps[cj])
                    if debug and qc == 0:
                        de = work.tile([128, 512], f32, tag="dbg_esum",
                                       name="dbg_esum", bufs=1)
                        nc.vector.tensor_copy(out=de, in_=esum_bf)
                        nc.sync.dma_start(out=dbg["dbg_esum"][:, :], in_=de)
                        dh = work.tile([128, 512], f32, tag="dbg_hoT",
                                       name="dbg_hoT", bufs=1)
                        nc.vector.tensor_copy(out=dh, in_=hoT8[0][:, 0, :])
                        nc.sync.dma_start(out=dbg["dbg_hoT"][:, :], in_=dh)
                        nc.sync.dma_start(out=dbg["dbg_inv"][:, :], in_=invb)
                    nc.leave_named_scope("attn_tail", scope_tail[0], False)

                    for co in range(CT):
                        ps = pp_proj.tile([128, 512], f32, tag="p_ps",
                                          name="pr_ps")
                        for pi in range(2):
                            nc.tensor.matmul(
                                out=ps,
                                lhsT=wp8[pi][:, :, co * 128:(co + 1) * 128],
                                rhs=hoT8[pi],
                                start=(pi == 0), stop=(pi == 1),
                                perf_mode=DR)
                        ot = work.tile([128, 512], f32, tag="ot", name="ot",
                                       bufs=2)
                        nc.vector.tensor_mul(out=ot, in0=ps, in1=invb)
                        nc.vector.tensor_scalar_add(out=ot, in0=ot,
                                                    scalar1=pbe[:, co:co + 1])
                        nc.vector.tensor_add(out=ot, in0=ot, in1=xr_tiles[co])
                        for oh in range(2):
                            nc.sync.dma_start(
                                out=out[co * 128:(co + 1) * 128,
                                        qc * 512 + oh * 256:
                                        qc * 512 + (oh + 1) * 256],
                                in_=ot[:, oh * 256:(oh + 1) * 256])
                return emit

            for qc in range(QC):
                qs = slice(qc * 512, (qc + 1) * 512)
                hoT_ps = [pp_acc.tile([128, 512], f32, tag="acc", name="acc")
                          for _ in range(CT)]
                esum = work.tile([128, 2, 512], f32, tag="esum", name="esum",
                                 bufs=2)
                # residual prefetch for this qc's tail
                xr_tiles = []
                for co in range(CT):
                    xr = work.tile([128, 512], f32, tag="xr", name="xr",
                                   bufs=6)
                    nc.sync.dma_start(out=xr,
                                      in_=Xr[co * 128:(co + 1) * 128, qs])
                    xr_tiles.append(xr)

                def es_pair(p):
                    ep = work.tile([128, 2, 512], f8, tag="es", name="es",
                                   bufs=4)
                    for half in range(2):
                        kt = 2 * p + half
                        s_ps = pp_sps.tile([128, 512], f32, tag="s_ps",
                                           name="s_ps")
                        with nc.named_scope("attn_s"):
                            for ci in range(CT):
                                nc.tensor.matmul(
                                    out=s_ps,
                                    lhsT=k_sb[ci][:, kt * 128:(kt + 1) * 128],
                                    rhs=q_sb[ci][:, qs],
                                    start=(ci == 0), stop=(ci == CT - 1))
                        nc.scalar.activation(out=ep[:, half, :], in_=s_ps,
                                             func=AF.Exp, scale=SCALE,
                                             bias=zsh_t)
                    return ep

                ep_cur = es_pair(0)
                ep_nxt = es_pair(1)
                # previous qc's tail hides under this qc's first s-matmuls
                if pend_tail[0] is not None:
                    pend_tail[0]()
                    pend_tail[0] = None
                for p in range(NP):
                    with nc.named_scope("attn_ho"):
                        for cj in range(CT):
                            nc.tensor.matmul(
                                out=hoT_ps[cj],
                                lhsT=v8[p][:, :, cj * 128:(cj + 1) * 128],
                                rhs=ep_cur,
                                start=(p == 0), stop=(p == NP - 1),
                                perf_mode=DR)
                    # softmax denominators: DVE f32 += fp8, flat pair adds
                    epf = ep_cur.rearrange("p two n -> p (two n)")
                    esf = esum.rearrange("p two n -> p (two n)")
                    if p == 0:
                        nc.vector.tensor_copy(out=esf, in_=epf)
                    else:
                        nc.vector.tensor_add(out=esf, in0=esf, in1=epf)
                    ep_cur = ep_nxt
                    if p + 2 < NP:
                        ep_nxt = es_pair(p + 2)
                pend_tail[0] = make_tail(qc, qs, hoT_ps, esum, xr_tiles)
            pend_tail[0]()

    nc.compile()
    return nc


def _get_nc():
    if "nc" not in _CACHE:
        _CACHE["nc"] = _build()
    return _CACHE["nc"]


def _prep_in_maps(X, gn_w, gn_b, wq, bq, wk, bk, wv, bv, wp, bp):
    import ml_dtypes
    bfl = ml_dtypes.bfloat16
    e4 = ml_dtypes.float8_e4m3

    X = np.ascontiguousarray(np.asarray(X, dtype=np.float32))
    f = lambda a: np.ascontiguousarray(np.asarray(a, dtype=np.float32))
    gn_w, gn_b, bq, bk, bv, bp = map(f, (gn_w, gn_b, bq, bk, bv, bp))
    wq, wk, wv, wp = map(f, (wq, wk, wv, wp))

    Xf = X.reshape(B, C, N)
    Xf_bf = Xf.astype(bfl)                       # [4, C, N]
    # fp8 X in DoubleRow pair layout: [pair, 128, (2, N)]
    X8v = Xf.reshape(B, 2, 2, 128, N).transpose(0, 1, 3, 2, 4)  # b,pair,p,i,n
    X8 = np.ascontiguousarray(X8v.reshape(B, 2, 128, 2 * N)).astype(e4)
    bpe = wp @ bv + bp  # bv folded through proj_out (softmax rows sum to 1)
    wqT = np.ascontiguousarray(wq.T).astype(bfl)
    wkT = np.ascontiguousarray(wk.T).astype(bfl)
    wpT = np.ascontiguousarray(wp.T).astype(bfl)
    wp8v = wp.T.reshape(2, 2, 128, C).transpose(0, 2, 1, 3)  # pair,p,i,cout
    wp8 = np.ascontiguousarray(wp8v.reshape(2, 128, 2 * C)).astype(e4)
    wvT = np.ascontiguousarray(wv.T)

    gmat = np.zeros((128, GPT), np.float32)
    for g in range(GPT):
        gmat[g * GSZ:(g + 1) * GSZ, g] = 1.0
    gmatT = np.ascontiguousarray(gmat.T)

    in_maps = []
    for core in range(8):
        bi, half = core // 2, core % 2
        q0 = half * NQ
        in_maps.append({
            "Xbf": Xf_bf[bi],
            "X8": X8[bi],
            "Xq": np.ascontiguousarray(Xf_bf[bi][:, q0:q0 + NQ]),
            "Xr": np.ascontiguousarray(Xf[bi][:, q0:q0 + NQ]),
            "wqT": wqT, "wkT": wkT, "wvT": wvT, "wpT": wpT, "wp8_d": wp8,
            "bq": bq, "bk": bk, "bpe": bpe, "gn_w": gn_w, "gn_b": gn_b,
            "gmat_d": gmat, "gmatT_d": gmatT,
        })
    return in_maps


_last_in_maps = None


def kernel(X, gn_w, gn_b, wq, bq, wk, bk, wv, bv, wp, bp):
    from concourse.bass_utils import run_bass_kernel_spmd

    global _last_in_maps
    in_maps = _prep_in_maps(X, gn_w, gn_b, wq, bq, wk, bk, wv, bv, wp, bp)
    _last_in_maps = in_maps
    nc = _get_nc()
    res = run_bass_kernel_spmd(nc, in_maps, list(range(8)))
    out = np.empty((B, C, N), np.float32)
    for core in range(8):
        bi, half = core // 2, core % 2
        out[bi][:, half * NQ:(half + 1) * NQ] = res.results[core]["out"]
    return out.reshape(B, C, H, W)


# revision 8
# speedup vs baseline: 1.6275x; 1.1865x over previous
"""AttnBlock (GroupNorm + single-head self-attention + residual) on 8 trn2 cores.

Problem: X [4, 512, 64, 64] f32. Per batch element: GroupNorm(32 groups), then
1x1-conv Q/K/V projections, softmax attention over n=h*w=4096 positions,
proj_out, residual add.

Sharding: 8 cores = 4 batch elements x 2 query-halves. Each core computes the
full GroupNorm + K/V for its batch element (duplicated within the pair) and
attention output for its 2048-query half.

v2 layout strategy (per core):
  X streams once from HBM (host pre-converted bf16 + fp8 copies) directly into
  resident SBUF tiles; GroupNorm stats run on the fp8 copy (2.1MB critical
  path).  GN is folded into the projection weights.  K/Q are bf16 [c, n] (full
  PE rate); V is fp8 e4m3 in DoubleRow pair layout [k, 2, c] so the PV matmul
  runs 256 keys per instruction AND produces Ho already transposed [c, q] for
  proj_out (no PE transposes).  Softmax: es = exp(S*scale - Z) quantized to
  e4m3 (Z=4; validated ~6e-3 rel err vs the 2e-2 budget).  Row sums come from
  a DVE f32 accumulation of the es tiles (no ones-matvec LDWEIGHTS); 1/sum is
  folded in AFTER proj_out via a rank-1 broadcast matmul, since proj is linear
  in the per-query scale.  Per-qc tails are emitted after the next qc's first
  attention matmuls so the tail's vector chain hides under PE work.
"""

import numpy as np

B, C, H, W = 4, 512, 64, 64
N = H * W            # 4096 keys per batch element
NQ = N // 2          # 2048 queries per core
CT = C // 128        # 4 channel tiles
NT = N // 128        # 32 key tiles
NP = NT // 2         # 16 key pair-tiles (DoubleRow)
QC = NQ // 512       # 4 query chunks of 512
GROUPS = 32
GPT = GROUPS // CT   # 8 groups per 128-channel tile
GSZ = C // GROUPS    # 16 channels per group
EPS = 1e-5
SCALE = float(C) ** -0.5
ZSHIFT = 4.0         # exp shift: es = exp(S*scale - Z); S*scale in ~[-7.3, 7.3]

_CACHE = {}


def _build(debug=False):
    from contextlib import ExitStack
    from concourse import bacc
    import concourse.mybir as mybir
    import concourse.tile as tile

    f32 = mybir.dt.float32
    f32r = mybir.dt.float32r
    bf16 = mybir.dt.bfloat16
    f8 = mybir.dt.float8e4
    AF = mybir.ActivationFunctionType
    DR = mybir.MatmulPerfMode.DoubleRow

    nc = bacc.Bacc()
    Xbf = nc.dram_tensor("Xbf", [C, N], bf16, kind="ExternalInput")
    X8 = nc.dram_tensor("X8", [2, 128, 2 * N], f8, kind="ExternalInput")
    Xq = nc.dram_tensor("Xq", [C, NQ], bf16, kind="ExternalInput")
    Xr = nc.dram_tensor("Xr", [C, NQ], f32, kind="ExternalInput")
    wT = {}
    for nm in ("wqT", "wkT", "wpT"):
        wT[nm] = nc.dram_tensor(nm, [C, C], bf16, kind="ExternalInput")
    wp8_d = nc.dram_tensor("wp8_d", [2, 128, 2 * C], f8, kind="ExternalInput")
    wT["wvT"] = nc.dram_tensor("wvT", [C, C], f32, kind="ExternalInput")
    vecs = {
        nm: nc.dram_tensor(nm, [C], f32, kind="ExternalInput")
        for nm in ("bq", "bk", "bpe", "gn_w", "gn_b")
    }
    gmat_d = nc.dram_tensor("gmat_d", [128, GPT], f32, kind="ExternalInput")
    gmatT_d = nc.dram_tensor("gmatT_d", [GPT, 128], f32, kind="ExternalInput")
    out = nc.dram_tensor("out", [C, NQ], f32, kind="ExternalOutput")
    pbe_d = nc.dram_tensor("pbe_d", [128, CT], f32, kind="Internal")
    dbg = {}
    if debug:
        for nm, shp in [("dbg_scbi", [128, 2 * CT]), ("dbg_q", [128, 512]),
                        ("dbg_k", [128, 512]), ("dbg_v", [128, 2 * 512]),
                        ("dbg_esum", [128, 512]),
                        ("dbg_hoT", [128, 512]), ("dbg_inv", [128, 512])]:
            dbg[nm] = nc.dram_tensor(nm, shp, f32, kind="ExternalOutput")

    def load_f32r(pool, stage_pool, dram_ap, shape, tag):
        """DMA f32 -> staging, DVE-convert -> f32r tile (real format change)."""
        st = stage_pool.tile(shape, f32, tag="ld_stage", name="ld_stage")
        nc.sync.dma_start(out=st, in_=dram_ap)
        t = pool.tile(shape, f32r, tag=tag, name=tag)
        nc.vector.tensor_copy(out=t, in_=st)
        return t

    with tile.TileContext(nc) as tc, ExitStack() as ctx:
        consts = ctx.enter_context(tc.tile_pool(name="consts", bufs=1))
        pp_acc = ctx.enter_context(tc.tile_pool(name="pp_acc", bufs=4, space="PSUM"))
        pp_sps = ctx.enter_context(tc.tile_pool(name="pp_sps", bufs=2, space="PSUM"))
        pp_proj = ctx.enter_context(tc.tile_pool(name="pp_proj", bufs=2, space="PSUM"))

        # persistent fp8 X cache in DoubleRow pair layout [128, 2, N]
        x8 = [consts.tile([128, 2, N], f8, tag=f"x8_{p}", name=f"x8_{p}")
              for p in range(2)]
        # persistent bf16 X cache [c-tile][128, N]
        x_bf = [consts.tile([128, N], bf16, tag=f"xbf{ci}", name=f"xbf{ci}")
                for ci in range(CT)]

        front_cm = tc.tile_pool(name="front", bufs=1)
        front = front_cm.__enter__()

        # tiny high-priority DMAs first: gn2 matrices + bias vectors
        with tc.tile_pool(name="cstage", bufs=2) as cstage:
            gmat = load_f32r(consts, cstage, gmat_d[:, :], [128, GPT], "gmat")
            gmatT = load_f32r(consts, cstage, gmatT_d[:, :], [GPT, 128], "gmatT")
        vt = {}
        for nm in ("bq", "bk", "bpe", "gn_w", "gn_b"):
            vt[nm] = consts.tile([128, CT], f32, tag=nm, name=nm)
            nc.sync.dma_start(
                out=vt[nm], in_=vecs[nm].rearrange("(c p) -> p c", p=128))

        # ---- pass A: stream X8 (fp8), run GroupNorm statistics on it ----
        # (emitted FIRST so these DMA triggers lead the queues)
        gst_cm = tc.tile_pool(name="gn_stats", bufs=2)
        gstats = gst_cm.__enter__()
        rowst_all = gstats.tile([128, CT, 2], f32r, tag="rowst", name="rowst")
        with nc.named_scope("gn"):
            for p2 in range(2):
                stats = [gstats.tile([128, 8, 6], f32, tag=f"bnst{i}",
                                     name=f"bnst{i}") for i in range(2)]
                for n8 in range(8):
                    sl = slice(n8 * 512, (n8 + 1) * 512)
                    eng = nc.gpsimd if (p2 * 8 + n8) % 2 else nc.sync
                    eng.dma_start(out=x8[p2][:, :, sl],
                                  in_=X8[p2, :, :].rearrange(
                                      "p (two n) -> p two n", two=2)[:, :, sl])
                    for i in range(2):
                        nc.vector.bn_stats(out=stats[i][:, n8, :],
                                           in_=x8[p2][:, i, sl])
                for i in range(2):
                    ci = 2 * p2 + i
                    mv = gstats.tile([128, 2], f32, tag="mv", name="mv")
                    nc.vector.bn_aggr(out=mv, in_=stats[i])
                    # rowstats = [mean, E[x^2]] ; E[x^2] = var + mean^2
                    nc.vector.tensor_copy(out=rowst_all[:, ci, 0:1],
                                          in_=mv[:, 0:1])
                    m2 = gstats.tile([128, 1], f32, tag="m2", name="m2")
                    nc.vector.tensor_mul(out=m2, in0=mv[:, 0:1], in1=mv[:, 0:1])
                    nc.vector.tensor_add(out=rowst_all[:, ci, 1:2],
                                         in0=mv[:, 1:2], in1=m2)

        # ---- early DMAs (split small, overlap stats) ----
        for ci in range(CT):
            rs = slice(ci * 128, (ci + 1) * 128)
            for n8 in range(8):
                sl = slice(n8 * 512, (n8 + 1) * 512)
                eng = nc.gpsimd if (ci * 8 + n8) % 2 else nc.sync
                eng.dma_start(out=x_bf[ci][:, sl], in_=Xbf[rs, sl])
        wq_sb = [consts.tile([128, C], bf16, tag=f"wq{ci}", name=f"wq{ci}")
                 for ci in range(CT)]
        wk_sb = [consts.tile([128, C], bf16, tag=f"wk{ci}", name=f"wk{ci}")
                 for ci in range(CT)]
        wp8 = [consts.tile([128, 2, C], f8, tag=f"wp8_{p}", name=f"wp8_{p}")
               for p in range(2)]
        wp_sb = [front.tile([128, C], bf16, tag=f"wp{ci}", name=f"wp{ci}")
                 for ci in range(CT)]
        for p in range(2):
            nc.scalar.dma_start(
                out=wp8[p].rearrange("p two n -> p (two n)"), in_=wp8_d[p, :, :])
        # wv stays f32 (staged) so the fold+fp8 conversion is single-rounding
        wv_st = [front.tile([128, C], f32, tag=f"wvst{ci}", name=f"wvst{ci}")
                 for ci in range(CT)]
        for ci in range(CT):
            rs = slice(ci * 128, (ci + 1) * 128)
            for hh in range(2):
                hs = slice(hh * 256, (hh + 1) * 256)
                nc.scalar.dma_start(out=wk_sb[ci][:, hs], in_=wT["wkT"][rs, hs])
                nc.scalar.dma_start(out=wq_sb[ci][:, hs], in_=wT["wqT"][rs, hs])
                nc.scalar.dma_start(out=wv_st[ci][:, hs], in_=wT["wvT"][rs, hs])
                nc.scalar.dma_start(out=wp_sb[ci][:, hs], in_=wT["wpT"][rs, hs])  # bf16 copy for pbe matvec only
        # query-half bf16 input for Q projection
        xq_bf = [front.tile([128, NQ], bf16, tag=f"xq{ci}", name=f"xq{ci}")
                 for ci in range(CT)]
        for ci in range(CT):
            rs = slice(ci * 128, (ci + 1) * 128)
            for hh in range(2):
                hs = slice(hh * 1024, (hh + 1) * 1024)
                nc.scalar.dma_start(out=xq_bf[ci][:, hs], in_=Xq[rs, hs])

        eps_t = consts.tile([128, 1], f32, tag="eps", name="eps")
        nc.vector.memset(eps_t, EPS)
        zsh_t = consts.tile([128, 1], f32, tag="zsh", name="zsh")
        nc.vector.memset(zsh_t, -ZSHIFT)
        ones_col = consts.tile([128, 1], bf16, tag="ones_c", name="ones_c")
        nc.vector.memset(ones_col, 1.0)
        ones_row = consts.tile([1, 128], bf16, tag="ones_r", name="ones_r")
        nc.vector.memset(ones_row, 1.0)

        # per-row GN affine: hn = x * sc_all[:,ci] + bi_all[:,ci]
        sc_all = consts.tile([128, CT], f32, tag="sc_all", name="sc_all")
        bi_all = consts.tile([128, CT], f32, tag="bi_all", name="bi_all")
        bi2 = consts.tile([128, CT, 2], bf16, tag="bi2", name="bi2")
        kb_sb = consts.tile([128, CT], f32, tag="kb_sb", name="kb_sb")
        qb_sb = consts.tile([128, CT], f32, tag="qb_sb", name="qb_sb")
        vb2 = consts.tile([128, CT, 2], bf16, tag="vb2", name="vb2")
        pbe = consts.tile([128, CT], f32, tag="pbe", name="pbe")

        with nc.named_scope("gn2"):
            # group-reduce 128 rows -> 8 groups -> broadcast, all ci at once
            gps = pp_sps.tile([GPT, CT, 2], f32, tag="s_ps", name="gps")
            nc.tensor.matmul(out=gps, lhsT=gmat,
                             rhs=rowst_all.rearrange("p c two -> p (c two)"),
                             start=True, stop=True)
            gsb = gstats.tile([GPT, CT * 2], f32r, tag="gsb", name="gsb")
            nc.vector.tensor_copy(out=gsb,
                                  in_=gps.rearrange("g c two -> g (c two)"))
            bps = pp_sps.tile([128, CT, 2], f32, tag="s_ps", name="bps")
            nc.tensor.matmul(out=bps, lhsT=gmatT, rhs=gsb,
                             start=True, stop=True)
            gstat = gstats.tile([128, CT, 2], f32, tag="gstat", name="gstat")
            nc.scalar.mul(out=gstat, in_=bps, mul=1.0 / GSZ)

            means = gstat[:, :, 0:1].rearrange("p c one -> p (c one)")
            m2s = gstat[:, :, 1:2].rearrange("p c one -> p (c one)")
            var = gstats.tile([128, CT], f32, tag="var", name="var")
            mm_ = gstats.tile([128, CT], f32, tag="mm_", name="mm_")
            nc.vector.tensor_mul(out=mm_, in0=means, in1=means)
            nc.vector.tensor_sub(out=var, in0=m2s, in1=mm_)
            # rstd = 1/sqrt(var + eps)
            nc.scalar.activation(out=var, in_=var, func=AF.Sqrt,
                                 bias=eps_t, scale=1.0)
            rstd = gstats.tile([128, CT], f32, tag="rstd", name="rstd")
            nc.vector.reciprocal(out=rstd, in_=var)
            # sc = rstd * gn_w ; bi = gn_b - mean * sc
            nc.vector.tensor_mul(out=sc_all, in0=rstd, in1=vt["gn_w"])
            msc = gstats.tile([128, CT], f32, tag="msc", name="msc")
            nc.vector.tensor_mul(out=msc, in0=means, in1=sc_all)
            nc.vector.tensor_sub(out=bi_all, in0=vt["gn_b"], in1=msc)
            for ci in range(CT):
                nc.vector.tensor_copy(
                    out=bi2[:, ci, :],
                    in_=bi_all[:, ci:ci + 1].to_broadcast((128, 2)))

        gst_cm.__exit__(None, None, None)

        def bias_matvec(w_sb, rhs2, add_vec, outname):
            """[128, CT] per-partition vector = w.T-chunks @ rhs2 (+add_vec)."""
            outt = consts.tile([128, CT], f32, tag=outname, name=outname)
            for co in range(CT):
                ps = pp_sps.tile([128, 2], f32, tag="s_ps", name="bv_ps")
                for ci in range(CT):
                    nc.tensor.matmul(
                        out=ps, lhsT=w_sb[ci][:, co * 128:(co + 1) * 128],
                        rhs=rhs2[:, ci, :],
                        start=(ci == 0), stop=(ci == CT - 1))
                if add_vec is not None:
                    nc.vector.tensor_add(out=outt[:, co:co + 1],
                                         in0=ps[:, 0:1],
                                         in1=add_vec[:, co:co + 1])
                else:
                    nc.vector.tensor_copy(out=outt[:, co:co + 1], in_=ps[:, 0:1])
            return outt

        # K bias matvec on UNFOLDED wk, then fold: these alone gate kproj
        kb = bias_matvec(wk_sb, bi2, vt["bk"], "kb_t")
        nc.vector.tensor_copy(out=kb_sb, in_=kb)
        for ci in range(CT):
            nc.vector.tensor_scalar_mul(out=wk_sb[ci], in0=wk_sb[ci],
                                        scalar1=sc_all[:, ci:ci + 1])

        # K lives in SBUF from projection straight through attention.
        k_sb = [consts.tile([128, N], bf16, tag=f"k{ci}", name=f"k{ci}")
                for ci in range(CT)]
        q_sb = [consts.tile([128, NQ], bf16, tag=f"q{co}", name=f"q{co}")
                for co in range(CT)]
        v8 = [consts.tile([128, 2, 512], f8, tag=f"v8_{p}", name=f"v8_{p}")
              for p in range(NP)]

        # ---- K projection (bf16): K[co, n] = sum_ci wkf[ci].T @ x_bf[ci] ----
        with nc.named_scope("kproj"):
            for e8 in range(8):
                ns = slice(e8 * 512, (e8 + 1) * 512)
                for co in range(CT):
                    ps = pp_proj.tile([128, 512], f32, tag="p_ps", name="k_ps")
                    for ci in range(CT):
                        nc.tensor.matmul(
                            out=ps, lhsT=wk_sb[ci][:, co * 128:(co + 1) * 128],
                            rhs=x_bf[ci][:, ns],
                            start=(ci == 0), stop=(ci == CT - 1))
                    nc.vector.tensor_scalar_add(out=k_sb[co][:, ns], in0=ps,
                                                scalar1=kb_sb[:, co:co + 1])

        # remaining matvecs/folds overlap kproj's PE work
        qb = bias_matvec(wq_sb, bi2, vt["bq"], "qb_t")
        nc.vector.tensor_copy(out=qb_sb, in_=qb)
        with tc.tile_pool(name="wvbf", bufs=1) as wvbfp:
            wv_bf = []
            for ci in range(CT):
                t = wvbfp.tile([128, C], bf16, tag=f"wvbf{ci}", name=f"wvbf{ci}")
                nc.vector.tensor_copy(out=t, in_=wv_st[ci])
                wv_bf.append(t)
            vb = bias_matvec(wv_bf, bi2, None, "vb_t")
            for ci in range(CT):
                nc.vector.tensor_copy(
                    out=vb2[:, ci, :],
                    in_=vb[:, ci:ci + 1].to_broadcast((128, 2)))
            pb = bias_matvec(wp_sb, vb2, vt["bpe"], "pb_t")
            nc.vector.tensor_copy(out=pbe, in_=pb)
        # pbe -> 4 bf16 row vectors via DRAM-transpose roundtrip (for the
        # rank-1 pbe (x) sums term folded into proj_out)
        nc.sync.dma_start(out=pbe_d[:, :], in_=pbe)
        pbe_rows = []
        for co in range(CT):
            r = consts.tile([1, 128], bf16, tag=f"pber{co}", name=f"pber{co}")
            st = consts.tile([1, 128], f32, tag=f"pbers{co}", name=f"pbers{co}")
            nc.sync.dma_start(
                out=st, in_=pbe_d[:, co:co + 1].rearrange("p one -> one p"))
            nc.vector.tensor_copy(out=r, in_=st)
            pbe_rows.append(r)
        for ci in range(CT):
            nc.vector.tensor_scalar_mul(out=wq_sb[ci], in0=wq_sb[ci],
                                        scalar1=sc_all[:, ci:ci + 1])
        wv8 = [consts.tile([128, 2, C], f8, tag=f"wv8_{p}", name=f"wv8_{p}")
               for p in range(2)]
        for ci in range(CT):
            nc.vector.tensor_scalar_mul(out=wv8[ci // 2][:, ci % 2, :],
                                        in0=wv_st[ci],
                                        scalar1=sc_all[:, ci:ci + 1])

        # ---- V projection (fp8 DoubleRow): V[nt, c] then store [k,2,c] ----
        with nc.named_scope("vproj"):
            for nt in range(NT):
                ps = pp_proj.tile([128, 512], f32, tag="p_ps", name="v_ps")
                for p in range(2):
                    nc.tensor.matmul(
                        out=ps,
                        lhsT=x8[p][:, :, nt * 128:(nt + 1) * 128],
                        rhs=wv8[p],
                        start=(p == 0), stop=(p == 1), perf_mode=DR)
                nc.scalar.copy(out=v8[nt // 2][:, nt % 2, :], in_=ps)
        # ---- Q projection (bf16) over this core's half ----
        with nc.named_scope("qproj"):
            for qn in range(QC):
                qs = slice(qn * 512, (qn + 1) * 512)
                for co in range(CT):
                    ps = pp_proj.tile([128, 512], f32, tag="p_ps", name="q_ps")
                    for ci in range(CT):
                        nc.tensor.matmul(
                            out=ps, lhsT=wq_sb[ci][:, co * 128:(co + 1) * 128],
                            rhs=xq_bf[ci][:, qs],
                            start=(ci == 0), stop=(ci == CT - 1))
                    nc.vector.tensor_scalar_add(out=q_sb[co][:, qs], in0=ps,
                                                scalar1=qb_sb[:, co:co + 1])

        front_cm.__exit__(None, None, None)

        if debug:
            dt_ = consts.tile([128, 2 * CT], f32, tag="dbg1", name="dbg1")
            nc.vector.tensor_copy(out=dt_[:, :CT], in_=sc_all)
            nc.vector.tensor_copy(out=dt_[:, CT:], in_=bi_all)
            nc.sync.dma_start(out=dbg["dbg_scbi"][:, :], in_=dt_)
            dq = consts.tile([128, 512], f32, tag="dbg_q", name="dbg_q")
            nc.vector.tensor_copy(out=dq, in_=q_sb[0][:, :512])
            nc.sync.dma_start(out=dbg["dbg_q"][:, :], in_=dq)
            dk = consts.tile([128, 512], f32, tag="dbg_k", name="dbg_k")
            nc.vector.tensor_copy(out=dk, in_=k_sb[0][:, :512])
            nc.sync.dma_start(out=dbg["dbg_k"][:, :], in_=dk)
            dv = consts.tile([128, 2 * 512], f32, tag="dbg_v", name="dbg_v")
            nc.vector.tensor_copy(
                out=dv, in_=v8[0].rearrange("p two n -> p (two n)"))
            nc.sync.dma_start(out=dbg["dbg_v"][:, :], in_=dv)

        # ---- attention ----
        with tc.tile_pool(name="work", bufs=2) as work:
            pend_tail = [None]

            def make_tail(qc, qs, hoT_ps, esum, xr_tiles):
                def emit():
                    scope_tail = nc.enter_named_scope("attn_tail", False)
                    hoT8 = [work.tile([128, 2, 512], f8, tag="hoT",
                                       name="hoT", bufs=3) for _ in range(2)]
                    for cj in range(CT):
                        nc.vector.tensor_copy(out=hoT8[cj // 2][:, cj % 2, :],
                                              in_=hoT_ps[cj])
                    esum_bf = work.tile([128, 512], bf16, tag="esum_bf",
                                        name="esum_bf", bufs=2)
                    nc.vector.tensor_add(out=esum_bf, in0=esum[:, 0, :],
                                         in1=esum[:, 1, :])
                    sums_ps = pp_proj.tile([1, 512], f32, tag="p_ps",
                                           name="sums")
                    nc.tensor.matmul(out=sums_ps, lhsT=ones_col, rhs=esum_bf,
                                     start=True, stop=True)
                    sums_bf = work.tile([1, 512], bf16, tag="sums_bf",
                                        name="sums_bf", bufs=2)
                    nc.vector.tensor_copy(out=sums_bf, in_=sums_ps)
                    sumb_ps = pp_proj.tile([128, 512], f32, tag="p_ps",
                                           name="sumb")
                    nc.tensor.matmul(out=sumb_ps, lhsT=ones_row, rhs=sums_bf,
                                     start=True, stop=True)
                    invb = work.tile([128, 512], f32, tag="invb", name="invb",
                                     bufs=2)
                    nc.vector.reciprocal(out=invb, in_=sumb_ps)
                    if debug and qc == 0:
                        de = work.tile([128, 512], f32, tag="dbg_esum",
                                       name="dbg_esum", bufs=1)
                        nc.vector.tensor_copy(out=de, in_=esum_bf)
                        nc.sync.dma_start(out=dbg["dbg_esum"][:, :], in_=de)
                        dh = work.tile([128, 512], f32, tag="dbg_hoT",
                                       name="dbg_hoT", bufs=1)
                        nc.vector.tensor_copy(out=dh, in_=hoT8[0][:, 0, :])
                        nc.sync.dma_start(out=dbg["dbg_hoT"][:, :], in_=dh)
                        nc.sync.dma_start(out=dbg["dbg_inv"][:, :], in_=invb)
                    nc.leave_named_scope("attn_tail", scope_tail[0], False)

                    for co in range(CT):
                        ps = pp_proj.tile([128, 512], f32, tag="p_ps",
                                          name="pr_ps")
                        for pi in range(2):
                            nc.tensor.matmul(
                                out=ps,
                                lhsT=wp8[pi][:, :, co * 128:(co + 1) * 128],
                                rhs=hoT8[pi],
                                start=(pi == 0), stop=(pi == 1),
                                perf_mode=DR)
                        # rank-1 pbe (x) sums: (proj + pbe*sums) * inv
                        # == proj*inv + pbe
                        nc.tensor.matmul(
                            out=ps, lhsT=pbe_rows[co], rhs=sums_bf,
                            start=False, stop=True, skip_group_check=True)
                        ot = work.tile([128, 512], f32, tag="ot", name="ot",
                                       bufs=2)
                        nc.vector.tensor_mul(out=ot, in0=ps, in1=invb)
                        nc.vector.tensor_add(out=ot, in0=ot, in1=xr_tiles[co])
                        for oh in range(2):
                            nc.sync.dma_start(
                                out=out[co * 128:(co + 1) * 128,
                                        qc * 512 + oh * 256:
                                        qc * 512 + (oh + 1) * 256],
                                in_=ot[:, oh * 256:(oh + 1) * 256])
                return emit

            for qc in range(QC):
                qs = slice(qc * 512, (qc + 1) * 512)
                hoT_ps = [pp_acc.tile([128, 512], f32, tag="acc", name="acc")
                          for _ in range(CT)]
                esum = work.tile([128, 2, 512], f32, tag="esum", name="esum",
                                 bufs=2)
                # residual prefetch for this qc's tail
                xr_tiles = []
                for co in range(CT):
                    xr = work.tile([128, 512], f32, tag="xr", name="xr",
                                   bufs=6)
                    nc.sync.dma_start(out=xr,
                                      in_=Xr[co * 128:(co + 1) * 128, qs])
                    xr_tiles.append(xr)

                def es_pair(p):
                    ep = work.tile([128, 2, 512], f8, tag="es", name="es",
                                   bufs=4)
                    for half in range(2):
                        kt = 2 * p + half
                        s_ps = pp_sps.tile([128, 512], f32, tag="s_ps",
                                           name="s_ps")
                        with nc.named_scope("attn_s"):
                            for ci in range(CT):
                                nc.tensor.matmul(
                                    out=s_ps,
                                    lhsT=k_sb[ci][:, kt * 128:(kt + 1) * 128],
                                    rhs=q_sb[ci][:, qs],
                                    start=(ci == 0), stop=(ci == CT - 1))
                        nc.scalar.activation(out=ep[:, half, :], in_=s_ps,
                                             func=AF.Exp, scale=SCALE,
                                             bias=zsh_t)
                    return ep

                ep_cur = es_pair(0)
                ep_nxt = es_pair(1)
                ep_nxt2 = es_pair(2)
                # previous qc's tail hides under this qc's first s-matmuls
                if pend_tail[0] is not None:
                    pend_tail[0]()
                    pend_tail[0] = None
                for p in range(NP):
                    with nc.named_scope("attn_ho"):
                        for cj in range(CT):
                            nc.tensor.matmul(
                                out=hoT_ps[cj],
                                lhsT=v8[p][:, :, cj * 128:(cj + 1) * 128],
                                rhs=ep_cur,
                                start=(p == 0), stop=(p == NP - 1),
                                perf_mode=DR)
                    # softmax denominators: DVE f32 += fp8, flat pair adds
                    epf = ep_cur.rearrange("p two n -> p (two n)")
                    esf = esum.rearrange("p two n -> p (two n)")
                    if p == 0:
                        nc.vector.tensor_copy(out=esf, in_=epf)
                    else:
                        nc.vector.tensor_add(out=esf, in0=esf, in1=epf)
                    ep_cur = ep_nxt
                    ep_nxt = ep_nxt2
                    if p + 3 < NP:
                        ep_nxt2 = es_pair(p + 3)
                pend_tail[0] = make_tail(qc, qs, hoT_ps, esum, xr_tiles)
            pend_tail[0]()

    nc.compile()
    return nc


def _get_nc():
    if "nc" not in _CACHE:
        _CACHE["nc"] = _build()
    return _CACHE["nc"]


def _prep_in_maps(X, gn_w, gn_b, wq, bq, wk, bk, wv, bv, wp, bp):
    import ml_dtypes
    bfl = ml_dtypes.bfloat16
    e4 = ml_dtypes.float8_e4m3

    X = np.ascontiguousarray(np.asarray(X, dtype=np.float32))
    f = lambda a: np.ascontiguousarray(np.asarray(a, dtype=np.float32))
    gn_w, gn_b, bq, bk, bv, bp = map(f, (gn_w, gn_b, bq, bk, bv, bp))
    wq, wk, wv, wp = map(f, (wq, wk, wv, wp))

    Xf = X.reshape(B, C, N)
    Xf_bf = Xf.astype(bfl)                       # [4, C, N]
    # fp8 X in DoubleRow pair layout: [pair, 128, (2, N)]
    X8v = Xf.reshape(B, 2, 2, 128, N).transpose(0, 1, 3, 2, 4)  # b,pair,p,i,n
    X8 = np.ascontiguousarray(X8v.reshape(B, 2, 128, 2 * N)).astype(e4)
    bpe = wp @ bv + bp  # bv folded through proj_out (softmax rows sum to 1)
    wqT = np.ascontiguousarray(wq.T).astype(bfl)
    wkT = np.ascontiguousarray(wk.T).astype(bfl)
    wpT = np.ascontiguousarray(wp.T).astype(bfl)
    wp8v = wp.T.reshape(2, 2, 128, C).transpose(0, 2, 1, 3)  # pair,p,i,cout
    wp8 = np.ascontiguousarray(wp8v.reshape(2, 128, 2 * C)).astype(e4)
    wvT = np.ascontiguousarray(wv.T)

    gmat = np.zeros((128, GPT), np.float32)
    for g in range(GPT):
        gmat[g * GSZ:(g + 1) * GSZ, g] = 1.0
    gmatT = np.ascontiguousarray(gmat.T)

    in_maps = []
    for core in range(8):
        bi, half = core // 2, core % 2
        q0 = half * NQ
        in_maps.append({
            "Xbf": Xf_bf[bi],
            "X8": X8[bi],
            "Xq": np.ascontiguousarray(Xf_bf[bi][:, q0:q0 + NQ]),
            "Xr": np.ascontiguousarray(Xf[bi][:, q0:q0 + NQ]),
            "wqT": wqT, "wkT": wkT, "wvT": wvT, "wpT": wpT, "wp8_d": wp8,
            "bq": bq, "bk": bk, "bpe": bpe, "gn_w": gn_w, "gn_b": gn_b,
            "gmat_d": gmat, "gmatT_d": gmatT,
        })
    return in_maps


_last_in_maps = None


def kernel(X, gn_w, gn_b, wq, bq, wk, bk, wv, bv, wp, bp):
    from concourse.bass_utils import run_bass_kernel_spmd

    global _last_in_maps
    in_maps = _prep_in_maps(X, gn_w, gn_b, wq, bq, wk, bk, wv, bv, wp, bp)
    _last_in_maps = in_maps
    nc = _get_nc()
    res = run_bass_kernel_spmd(nc, in_maps, list(range(8)))
    out = np.empty((B, C, N), np.float32)
    for core in range(8):
        bi, half = core // 2, core % 2
        out[bi][:, half * NQ:(half + 1) * NQ] = res.results[core]["out"]
    return out.reshape(B, C, H, W)
